# revision 1
# baseline (speedup 1.0000x reference)
"""Trainium2 Bass kernel for nn_BipartiteGCN (6-layer bipartite GCN,
200K read nodes, 50K intron nodes, 2M random edges).

Strategy (8 NeuronCores, SPMD):
 - Shard edges by READ-node range: core c owns reads [25000c, 25000(c+1))
   and ALL edges incident to them.  Read-side aggregations are then exact
   and local; intron-side aggregations produce partials that are combined
   with ReduceScatter (+AllGather of the processed feature tables).
 - Per layer, node features live in DRAM tables of 256B bf16 rows
   (feature dim padded to 128).  The random side of each layer's
   gather/scatter is handled by batched SWDGE dma_gather (256B rows,
   int16 indices, tables kept < 32768 rows by splitting the intron table
   into two halves); the sorted side is handled by one-hot matmuls on the
   TensorEngine accumulating segment sums in PSUM.
 - One-hots are built on VectorE as bf16 is_equal against a pre-built
   replicated iota, 32 blocks (4096 edges) per instruction.
 - Degree vectors (pure functions of the integer edge lists, like the
   sort/padding metadata itself) are computed host-side as bincounts; the
   float math 1/sqrt(max(deg,1)) and everything downstream runs on device.
 - D^-1/2 scalings are folded into table construction (source side) and
   into per-partition activation scales at PSUM-drain time (dest side,
   using relu(s*x) = s*relu(x) for s>0).
"""

import os
import sys
import numpy as np
import ml_dtypes

sys.path.insert(0, "/opt/trn_rl_repo")

from contextlib import ExitStack

import concourse.bass as bass
import concourse.tile as tile
import concourse.mybir as mybir
from concourse import bacc, bass_utils
from concourse.masks import make_identity

P = 128
NCORES = 8
N_READ = 200000
N_INTRON = 50000
N_EDGES = 2000000

R_LOC = 25000          # real reads per core
R_PAD = 25088          # padded read slots per core (196 chunks)
NWIN = 196             # read windows (chunks of 128)
ISLOT = 50176          # padded intron slots (392 chunks)
NCH_A = 392            # intron chunks
IHALF = 25088          # intron slots per half
RSROWS = 25600         # RS payload rows per half (divisible: 3200/rank)
SHARD = 3200           # rows per rank after RS
NSC = 25               # shard chunks of 128 rows
SENT_A = R_PAD - 1     # sentinel gather row in TA tables (zero row)
SENT_B = IHALF         # sentinel gather row in TB tables (zero row)
GRP = 32               # blocks per one-hot instruction
GB = 48                # blocks per dma_gather call

bf16 = mybir.dt.bfloat16
f32 = mybir.dt.float32
i16 = mybir.dt.int16

_BUILT = {}
LAST_RESULTS = None


# ----------------------------------------------------------------------
# host-side prep
# ----------------------------------------------------------------------

def _wrap_idx(a):
    """[N] -> [128, N//16] int16 wrapped layout for dma_gather."""
    w = a.astype(np.int16).reshape(-1, 16).T
    return np.ascontiguousarray(np.tile(w, (8, 1)))


def _pmaj(a, nblk):
    """flat [nblk*128] -> [128, nblk] partition-major (e -> [e%128, e//128])"""
    return np.ascontiguousarray(a.reshape(nblk, P).T)


def _build_order(gid, payload_idx, payload_rel, n_groups, blk):
    """Scatter edges (sorted by group id) into padded per-group block slots.

    gid: [n] group id per edge;  blk: [n_groups] blocks per group (shared
    across cores).  Returns (gidx_pad int64, rel_pad int64) of length
    sum(blk)*128 with sentinel -1 in unfilled slots."""
    order = np.argsort(gid, kind="stable")
    gs = gid[order]
    cnt = np.bincount(gid, minlength=n_groups)
    raw_start = np.concatenate([[0], np.cumsum(cnt)[:-1]])
    pad_start = np.concatenate([[0], np.cumsum(blk)[:-1]]) * P
    n = gid.shape[0]
    pos_in_g = np.arange(n) - np.repeat(raw_start, cnt)
    pos = pad_start[gs] + pos_in_g
    tot = int(blk.sum()) * P
    gidx = np.full(tot, -1, np.int64)
    rel = np.zeros(tot, np.int64)
    gidx[pos] = payload_idx[order]
    rel[pos] = payload_rel[order]
    return gidx, rel


def _prep(inputs):
    src = np.asarray(inputs["edge_src"]).astype(np.int64)
    dst = np.asarray(inputs["edge_dst"]).astype(np.int64)
    h_read = np.asarray(inputs["h_read"]).astype(np.float32)

    core = src // R_LOC
    deg_i_glob = np.bincount(dst, minlength=ISLOT).astype(np.float32)

    per_core = []
    cntA = np.zeros((NCORES, NCH_A), np.int64)
    cntB = np.zeros((NCORES, 2 * NWIN), np.int64)
    for c in range(NCORES):
        m = core == c
        s = src[m] - c * R_LOC
        d = dst[m]
        chA = d // P                       # order-A group: intron chunk
        half = d // IHALF
        gB = half * NWIN + (s // P)        # order-B group: (half, window)
        cntA[c] = np.bincount(chA, minlength=NCH_A)
        cntB[c] = np.bincount(gB, minlength=2 * NWIN)
        per_core.append((s, d, chA, gB, half))

    blkA = np.maximum(1, -(-cntA.max(axis=0) // P))
    blkB = np.maximum(1, -(-cntB.max(axis=0) // P))
    nblkA = int(blkA.sum())
    nblkB = int(blkB.sum())

    in_maps = []
    for c in range(NCORES):
        s, d, chA, gB, half = per_core[c]
        gA, relA = _build_order(chA, s, d % P, NCH_A, blkA)
        gA[gA < 0] = SENT_A
        gBi, relB = _build_order(gB, d - half * IHALF, s % P, 2 * NWIN, blkB)
        gBi[gBi < 0] = SENT_B

        deg_r = np.bincount(s, minlength=R_PAD).astype(np.float32)
        # my shard slice of intron degrees: col j = h*NSC + cc;
        # slot = h*IHALF + c*SHARD + cc*128 + p  (junk rows -> 1.0)
        deg_my = np.ones((P, 2 * NSC), np.float32)
        for h in range(2):
            rows = c * SHARD + np.arange(NSC * P)
            valid = rows < IHALF
            slots = h * IHALF + rows
            v = np.ones(NSC * P, np.float32)
            v[valid] = deg_i_glob[slots[valid]]
            deg_my[:, h * NSC:(h + 1) * NSC] = v.reshape(NSC, P).T

        hrT = np.zeros((10, R_PAD), np.float32)
        hrT[:, :R_LOC] = h_read[c * R_LOC:(c + 1) * R_LOC].T

        bf = ml_dtypes.bfloat16
        im = {
            "gidxA": _wrap_idx(gA),
            "drelA": _pmaj(relA, nblkA).astype(bf),
            "gidxB": _wrap_idx(gBi),
            "drelB": _pmaj(relB, nblkB).astype(bf),
            "hrT": hrT,
            "degr": np.ascontiguousarray(
                deg_r.reshape(NWIN, P).T).astype(np.float32),
            "degi": deg_my,
            "w0": np.asarray(inputs["W0"]).astype(np.float32),
            "wext1": np.concatenate(
                [np.asarray(inputs["W1"]),
                 np.asarray(inputs["b1"])[None, :]], 0).astype(bf),
            "atts": np.asarray(inputs["atts"]).reshape(1, 6)
                      .astype(np.float32),
            "fcw": np.asarray(inputs["fc_w"]).astype(bf),
            "fcb": np.asarray(inputs["fc_b"]).reshape(1, 2).astype(bf),
        }
        for l in (2, 3, 4, 5):
            im[f"w{l}"] = np.asarray(inputs[f"W{l}"]).astype(bf)
            im[f"b{l}"] = np.asarray(inputs[f"b{l}"]).reshape(1, P).astype(bf)
        in_maps.append(im)

    meta = (tuple(int(x) for x in blkA), tuple(int(x) for x in blkB))
    return in_maps, meta


# ----------------------------------------------------------------------
# device program
# ----------------------------------------------------------------------

def _build(meta):
    blkA, blkB = (np.array(meta[0]), np.array(meta[1]))
    nblkA, nblkB = int(blkA.sum()), int(blkB.sum())

    nc = bacc.Bacc("TRN2", target_bir_lowering=False, debug=False,
                   num_devices=NCORES)

    # --- I/O ---
    t_gidxA = nc.dram_tensor("gidxA", [P, nblkA * 8], i16, kind="ExternalInput")
    t_drelA = nc.dram_tensor("drelA", [P, nblkA], bf16, kind="ExternalInput")
    t_gidxB = nc.dram_tensor("gidxB", [P, nblkB * 8], i16, kind="ExternalInput")
    t_drelB = nc.dram_tensor("drelB", [P, nblkB], bf16, kind="ExternalInput")
    t_hrT = nc.dram_tensor("hrT", [10, R_PAD], f32, kind="ExternalInput")
    t_degr = nc.dram_tensor("degr", [P, NWIN], f32, kind="ExternalInput")
    t_degi = nc.dram_tensor("degi", [P, 2 * NSC], f32, kind="ExternalInput")
    t_w0 = nc.dram_tensor("w0", [10, 64], f32, kind="ExternalInput")
    t_wext1 = nc.dram_tensor("wext1", [65, P], bf16, kind="ExternalInput")
    t_w = {l: nc.dram_tensor(f"w{l}", [P, P], bf16, kind="ExternalInput")
           for l in (2, 3, 4, 5)}
    t_b = {l: nc.dram_tensor(f"b{l}", [1, P], bf16, kind="ExternalInput")
           for l in (2, 3, 4, 5)}
    t_fcw = nc.dram_tensor("fcw", [P, 2], bf16, kind="ExternalInput")
    t_fcb = nc.dram_tensor("fcb", [1, 2], bf16, kind="ExternalInput")
    t_atts = nc.dram_tensor("atts", [1, 6], f32, kind="ExternalInput")
    t_out = nc.dram_tensor("out", [R_PAD, 2], f32, kind="ExternalOutput")

    groups = [list(range(NCORES))]

    with tile.TileContext(nc) as tc, ExitStack() as ctx:
        cst = ctx.enter_context(tc.tile_pool(name="cst", bufs=1))
        sb = ctx.enter_context(tc.tile_pool(name="sb", bufs=2))
        gpool = ctx.enter_context(tc.tile_pool(name="gp", bufs=3))
        ohpool = ctx.enter_context(tc.tile_pool(name="oh", bufs=2))
        ps1p = ctx.enter_context(tc.tile_pool(name="ps1", bufs=3, space="PSUM"))
        ps2p = ctx.enter_context(tc.tile_pool(name="ps2", bufs=2, space="PSUM"))
        ps3p = ctx.enter_context(tc.tile_pool(name="ps3", bufs=2, space="PSUM"))
        dram = ctx.enter_context(tc.tile_pool(name="dr", bufs=1, space="DRAM"))

        # ---------- constants ----------
        iota_i = cst.tile([P, P], mybir.dt.int32)
        nc.gpsimd.iota(iota_i[:], pattern=[[1, P]], base=0,
                       channel_multiplier=0)
        iota_bf = cst.tile([P, P], bf16)
        nc.vector.tensor_copy(iota_bf[:], iota_i[:])
        iota_rep = cst.tile([P, P, GRP], bf16)
        nc.vector.tensor_copy(
            iota_rep[:], iota_bf[:].unsqueeze(2).to_broadcast([P, P, GRP]))

        ident_f = cst.tile([P, P], f32)
        make_identity(nc, ident_f[:])
        ident_b = cst.tile([P, P], bf16)
        nc.vector.tensor_copy(ident_b[:], ident_f[:])

        ones_f = cst.tile([1, P], f32)
        nc.vector.memset(ones_f[:], 1.0)
        ones_b = cst.tile([1, P], bf16)
        nc.vector.memset(ones_b[:], 1.0)

        zero_sb = cst.tile([P, 4, P], bf16)
        nc.vector.memset(zero_sb[:], 0.0)

        # weights
        w0_sb = cst.tile([10, 64], f32)
        nc.sync.dma_start(w0_sb[:], t_w0[:])
        wext1_sb = cst.tile([65, P], bf16)
        nc.sync.dma_start(wext1_sb[:], t_wext1[:])
        w_sb, b_sb = {}, {}
        for l in (2, 3, 4, 5):
            w_sb[l] = cst.tile([P, P], bf16, tag=f"w{l}", name=f"w{l}sb")
            nc.sync.dma_start(w_sb[l][:], t_w[l][:])
            b_sb[l] = cst.tile([1, P], bf16, tag=f"b{l}", name=f"b{l}sb")
            nc.sync.dma_start(b_sb[l][:], t_b[l][:])
        fcw_sb = cst.tile([P, 2], bf16)
        nc.sync.dma_start(fcw_sb[:], t_fcw[:])
        fcb_sb = cst.tile([1, 2], bf16)
        nc.sync.dma_start(fcb_sb[:], t_fcb[:])

        # gates: sigmoid(atts) replicated to 128 partitions
        atts_sb = cst.tile([1, 6], f32)
        nc.sync.dma_start(atts_sb[:], t_atts[:])
        sg = cst.tile([1, 6], f32)
        nc.scalar.activation(sg[:], atts_sb[:],
                             mybir.ActivationFunctionType.Sigmoid)
        ps_g = ps3p.tile([P, 6], f32, tag="p3")
        nc.tensor.matmul(ps_g[:], lhsT=ones_f[:], rhs=sg[:],
                         start=True, stop=True)
        g_rep = cst.tile([P, 6], f32)
        nc.scalar.copy(g_rep[:], ps_g[:])

        # rs_r = 1/sqrt(max(deg_r,1)); per-layer drain scales
        degr_sb = cst.tile([P, NWIN], f32)
        nc.sync.dma_start(degr_sb[:], t_degr[:])
        rs_r = cst.tile([P, NWIN], f32)
        nc.vector.tensor_scalar_max(rs_r[:], degr_sb[:], 1.0)
        nc.scalar.sqrt(rs_r[:], rs_r[:])
        nc.vector.reciprocal(rs_r[:], rs_r[:])
        rgr = {}
        for l in (1, 3, 5):
            rgr[l] = cst.tile([P, NWIN], f32, tag=f"rgr{l}", name=f"rgr{l}")
            if l == 5:
                # last conv layer: no outer rs_r fold -> scale = rs_r * g5
                nc.vector.tensor_copy(rgr[l][:], rs_r[:])
            else:
                nc.vector.tensor_tensor(out=rgr[l][:], in0=rs_r[:],
                                        in1=rs_r[:], op=mybir.AluOpType.mult)
            nc.vector.tensor_tensor(
                out=rgr[l][:], in0=rgr[l][:],
                in1=g_rep[:, l:l + 1].to_broadcast([P, NWIN]),
                op=mybir.AluOpType.mult)

        degi_sb = cst.tile([P, 2 * NSC], f32)
        nc.sync.dma_start(degi_sb[:], t_degi[:])
        rs_i = cst.tile([P, 2 * NSC], f32)
        nc.vector.tensor_scalar_max(rs_i[:], degi_sb[:], 1.0)
        nc.scalar.sqrt(rs_i[:], rs_i[:])
        nc.vector.reciprocal(rs_i[:], rs_i[:])
        r2g = {}
        for l in (0, 2, 4):
            r2g[l] = cst.tile([P, 2 * NSC], f32, tag=f"r2g{l}", name=f"r2g{l}")
            nc.vector.tensor_tensor(out=r2g[l][:], in0=rs_i[:], in1=rs_i[:],
                                    op=mybir.AluOpType.mult)
            nc.vector.tensor_tensor(
                out=r2g[l][:], in0=r2g[l][:],
                in1=g_rep[:, l:l + 1].to_broadcast([P, 2 * NSC]),
                op=mybir.AluOpType.mult)

        # uT staging buffers with a fixed ones row (fin=64 path)
        uT65 = [cst.tile([65, P], bf16, tag=f"uT65_{i}", name=f"uT65_{i}")
                for i in range(2)]
        for t in uT65:
            nc.vector.memset(t[:], 1.0)
        uT128 = [cst.tile([P, P], bf16, tag=f"uT128_{i}", name=f"uT128_{i}")
                 for i in range(2)]

        # big shared buffer: z0 staging / order-B aggregation
        bigbuf = cst.tile([P, NWIN, P], bf16, tag="bigbuf")

        # DRAM tables & collective buffers
        TA = [dram.tile([R_PAD, P], bf16, tag=f"TA{k}", name=f"TA{k}")
              for k in range(3)]
        TB = {}
        rsin, rsout, zsh = {}, {}, {}
        for l in (0, 2, 4):
            TB[l] = [dram.tile([RSROWS, P], bf16, tag=f"TB{l}_{h}",
                               name=f"TB{l}_{h}",
                               addr_space="Shared") for h in range(2)]
            rsin[l] = [dram.tile([RSROWS, P], bf16, tag=f"rsin{l}_{h}",
                                 name=f"rsin{l}_{h}")
                       for h in range(2)]
            rsout[l] = [dram.tile([SHARD, P], bf16, tag=f"rso{l}_{h}",
                                  name=f"rso{l}_{h}")
                        for h in range(2)]
            zsh[l] = [dram.tile([SHARD, P], bf16, tag=f"zsh{l}_{h}",
                                name=f"zsh{l}_{h}")
                      for h in range(2)]
            for h in range(2):
                nc.sync.dma_start(
                    rsin[l][h][IHALF:RSROWS, :]
                    .rearrange("(c p) f -> p c f", p=P),
                    zero_sb[:])

        # ---------- z0 = (h_read * rs_r) @ W0  ->  TA[0] ----------
        nc.vector.memset(bigbuf[:], 0.0)
        PIECE = 16
        for p0 in range(0, NWIN, PIECE):
            pw = min(PIECE, NWIN - p0)
            hrp = sb.tile([10, pw * P], f32, tag="hrp")
            nc.sync.dma_start(hrp[:], t_hrT[:, p0 * P:(p0 + pw) * P])
            for wl in range(pw):
                w = p0 + wl
                psz = ps3p.tile([P, 64], f32, tag="p3")
                nc.tensor.matmul(psz[:], lhsT=hrp[:, wl * P:(wl + 1) * P],
                                 rhs=w0_sb[:], start=True, stop=True)
                nc.scalar.mul(bigbuf[:, w, :64], psz[:], rs_r[:, w:w + 1])
        nc.sync.dma_start(
            TA[0][:].rearrange("(w p) f -> p w f", p=P), bigbuf[:])

        # ---------- pass machinery ----------
        def gather_plan(blk, half_split_blocks):
            """split blocks into dma_gather calls of <=GB blocks, not
            crossing the half boundary (in block index space)."""
            calls = []
            for lo, hi in half_split_blocks:
                b = lo
                while b < hi:
                    n = min(GB, hi - b)
                    calls.append((b, n))
                    b += n
            return calls

        def scatter_pass(idx_dram, drel_dram, nblk, blk, tables, on_group):
            """Generic pass: gather + one-hot + psum accumulate per group.

            tables: list of (in_ap, blocks_lo, blocks_hi) gather sources.
            on_group(g, ps1): consume the accumulated psum for group g.
            """
            idx_sb = cst.tile([P, max(nblkA, nblkB) * 8], i16, tag="idxsb")
            nc.sync.dma_start(idx_sb[:, :nblk * 8], idx_dram[:])
            drel_sb = cst.tile([P, max(nblkA, nblkB)], bf16, tag="drelsb")
            nc.sync.dma_start(drel_sb[:, :nblk], drel_dram[:])

            calls = gather_plan(blk, [(lo, hi) for _, lo, hi in tables])
            tbl_of_call = {}
            for ci, (b0, nb) in enumerate(calls):
                for ap, lo, hi in tables:
                    if lo <= b0 < hi:
                        tbl_of_call[ci] = ap
            call_of_block = {}
            for ci, (b0, nb) in enumerate(calls):
                for b in range(b0, b0 + nb):
                    call_of_block[b] = ci

            gtiles, ohtiles = {}, {}
            starts = np.concatenate([[0], np.cumsum(blk)]).astype(int)
            ngrp = len(blk)
            for g in range(ngrp):
                ps1 = ps1p.tile([P, P], f32, tag="p1")
                for b in range(starts[g], starts[g + 1]):
                    ci = call_of_block[b]
                    if ci not in gtiles:
                        b0, nb = calls[ci]
                        gt = gpool.tile([P, nb, P], bf16, tag="gbuf")
                        nc.gpsimd.dma_gather(
                            gt[:], tbl_of_call[ci], idx_sb[:, b0 * 8:
                                                          (b0 + nb) * 8],
                            nb * P, nb * P, P, single_packet=False)
                        gtiles[ci] = gt
                        # drop old refs so pool slots rotate
                        for k in list(gtiles):
                            if k < ci - 2:
                                del gtiles[k]
                    oi = b // GRP
                    if oi not in ohtiles:
                        o0 = oi * GRP
                        on = min(GRP, nblk - o0)
                        oh = ohpool.tile([P, P, on], bf16, tag="oh")
                        nc.vector.tensor_tensor(
                            out=oh[:],
                            in0=drel_sb[:, o0:o0 + on].unsqueeze(1)
                                .to_broadcast([P, P, on]),
                            in1=iota_rep[:, :, :on],
                            op=mybir.AluOpType.is_equal)
                        ohtiles[oi] = oh
                        for k in list(ohtiles):
                            if k < oi - 1:
                                del ohtiles[k]
                    b0, nb = calls[ci]
                    nc.tensor.matmul(
                        ps1[:],
                        lhsT=ohtiles[oi][:, :, b - oi * GRP],
                        rhs=gtiles[ci][:, b - b0, :],
                        start=(b == starts[g]),
                        stop=(b == starts[g + 1] - 1))
                on_group(g, ps1)

        # ---------- intron-side (order A) pass: l in {0,2,4} ----------
        def a_pass(l, ta_ap):
            astage = [None]

            def shard_and_ag(h):
                nc.gpsimd.collective_compute(
                    "ReduceScatter", mybir.AluOpType.add,
                    replica_groups=groups,
                    ins=[rsin[l][h][:].opt()], outs=[rsout[l][h][:].opt()])
                zstage = None
                for cc in range(NSC):
                    sh_in = sb.tile([P, P], bf16, tag="shin")
                    nc.sync.dma_start(
                        sh_in[:], rsout[l][h][cc * P:(cc + 1) * P, :])
                    col = h * NSC + cc
                    if cc % 4 == 0:
                        zstage = sb.tile([P, 4, P], bf16, tag="zst")
                    if l == 0:
                        nc.scalar.activation(
                            zstage[:, cc % 4, :], sh_in[:],
                            mybir.ActivationFunctionType.Relu,
                            scale=r2g[l][:, col:col + 1])
                    else:
                        ps2 = ps2p.tile([P, P], bf16, tag="p2")
                        nc.tensor.transpose(ps2[:], sh_in[:], ident_b[:])
                        uT = uT128[cc % 2]
                        nc.scalar.copy(uT[:], ps2[:])
                        ps3 = ps3p.tile([P, P], f32, tag="p3")
                        nc.tensor.matmul(ps3[:], lhsT=uT[:], rhs=w_sb[l][:],
                                         start=True, stop=False)
                        nc.tensor.matmul(ps3[:], lhsT=ones_b[:],
                                         rhs=b_sb[l][:],
                                         start=False, stop=True)
                        nc.scalar.activation(
                            zstage[:, cc % 4, :], ps3[:],
                            mybir.ActivationFunctionType.Relu,
                            scale=r2g[l][:, col:col + 1])
                    if cc % 4 == 3 or cc == NSC - 1:
                        c0 = cc - cc % 4
                        nc.sync.dma_start(
                            zsh[l][h][c0 * P:(cc + 1) * P, :]
                            .rearrange("(c p) f -> p c f", p=P),
                            zstage[:, :cc % 4 + 1, :])
                # NOTE: TB tail rows (>= IHALF, incl. the sentinel row) are
                # zero because the rsin tail is zeroed and biases are zero.
                nc.gpsimd.collective_compute(
                    "AllGather", mybir.AluOpType.bypass,
                    replica_groups=groups,
                    ins=[zsh[l][h][:].opt()], outs=[TB[l][h][:].opt()])

            def on_group(ch, ps1):
                if ch % 4 == 0:
                    astage[0] = sb.tile([P, 4, P], bf16, tag="ast", name="ast")
                nc.scalar.copy(astage[0][:, ch % 4, :], ps1[:])
                if ch % 4 == 3:
                    h, chl = ch // NWIN, (ch - ch // NWIN * NWIN)
                    c0 = chl - 3
                    nc.sync.dma_start(
                        rsin[l][h][c0 * P:(chl + 1) * P, :]
                        .rearrange("(c p) f -> p c f", p=P), astage[0][:])

            scatter_pass(t_gidxA[:], t_drelA[:], nblkA, blkA,
                         [(ta_ap, 0, nblkA)], on_group)
            # SWDGE gathers in flight concurrently with ncfw collectives
            # deadlock on this stack -- hard-serialize the pass tail.
            tc.strict_bb_all_engine_barrier()
            shard_and_ag(0)
            shard_and_ag(1)
            tc.strict_bb_all_engine_barrier()

        # ---------- read-side (order B) pass: l in {1,3,5} ----------
        bstartsB = np.concatenate([[0], np.cumsum(blkB)]).astype(int)
        half_split_B = int(bstartsB[NWIN])

        def b_pass(l, out_sink):
            def post_window(w):
                fin = 64 if l == 1 else P
                ps2 = ps2p.tile([fin, P], bf16, tag="p2")
                nc.tensor.transpose(ps2[:], bigbuf[:, w, :fin], ident_b[:])
                if l == 1:
                    uT = uT65[w % 2]
                    nc.scalar.copy(uT[:64, :], ps2[:])
                    ps3 = ps3p.tile([P, P], f32, tag="p3")
                    nc.tensor.matmul(ps3[:], lhsT=uT[:], rhs=wext1_sb[:],
                                     start=True, stop=True)
                else:
                    uT = uT128[w % 2]
                    nc.scalar.copy(uT[:], ps2[:])
                    ps3 = ps3p.tile([P, P], f32, tag="p3")
                    nc.tensor.matmul(ps3[:], lhsT=uT[:], rhs=w_sb[l][:],
                                     start=True, stop=False)
                    nc.tensor.matmul(ps3[:], lhsT=ones_b[:], rhs=b_sb[l][:],
                                     start=False, stop=True)
                out_sink(w, ps3)

            def on_group(g, ps1):
                h, w = g // NWIN, g % NWIN
                if h == 0:
                    nc.scalar.copy(bigbuf[:, w, :], ps1[:])
                else:
                    nc.vector.tensor_tensor(
                        out=bigbuf[:, w, :], in0=ps1[:], in1=bigbuf[:, w, :],
                        op=mybir.AluOpType.add)
                    post_window(w)

            tabs = [(TB[l - 1][0][:], 0, half_split_B),
                    (TB[l - 1][1][:], half_split_B, nblkB)]
            scatter_pass(t_gidxB[:], t_drelB[:], nblkB, blkB, tabs, on_group)

        # L0
        a_pass(0, TA[0][:])

        # L1 -> TA[1]
        zr_stage = [None]

        def sink_l1(w, ps3, l=1, k=1):
            if w % 4 == 0:
                zr_stage[0] = sb.tile([P, 4, P], bf16, tag="zrst", name="zrst")
            nc.scalar.activation(zr_stage[0][:, w % 4, :], ps3[:],
                                 mybir.ActivationFunctionType.Relu,
                                 scale=rgr[l][:, w:w + 1])
            if w % 4 == 3:
                c0 = w - 3
                nc.sync.dma_start(
                    TA[k][c0 * P:(w + 1) * P, :]
                    .rearrange("(c p) f -> p c f", p=P), zr_stage[0][:])

        b_pass(1, sink_l1)
        a_pass(2, TA[1][:])
        b_pass(3, lambda w, ps3: sink_l1(w, ps3, l=3, k=2))
        a_pass(4, TA[2][:])

        # L5 -> fc -> out
        out_sb = cst.tile([P, NWIN, 2], f32, tag="outsb")

        def sink_l5(w, ps3):
            h5 = sb.tile([P, P], f32, tag="h5")
            nc.scalar.activation(h5[:], ps3[:],
                                 mybir.ActivationFunctionType.Relu,
                                 scale=rgr[5][:, w:w + 1])
            ps2b = ps2p.tile([P, P], f32, tag="p2")
            nc.tensor.transpose(ps2b[:], h5[:], ident_f[:])
            h5T = uT128[w % 2]
            nc.scalar.copy(h5T[:], ps2b[:])
            psf = ps3p.tile([P, 2], f32, tag="p3")
            nc.tensor.matmul(psf[:], lhsT=h5T[:], rhs=fcw_sb[:],
                             start=True, stop=False)
            nc.tensor.matmul(psf[:], lhsT=ones_b[:], rhs=fcb_sb[:],
                             start=False, stop=True)
            nc.scalar.copy(out_sb[:, w, :], psf[:])

        b_pass(5, sink_l5)
        nc.sync.dma_start(
            t_out[:].rearrange("(w p) c -> p w c", p=P), out_sb[:])

    nc.compile()
    return nc


# ----------------------------------------------------------------------
# entry point
# ----------------------------------------------------------------------

def _ensure_ntff_hook():
    """Install the axon NTFF profiling hook shim if the image's antenv
    lacks the axon_hooks module (profiling-only; safe to fail)."""
    try:
        from antenv.axon_hooks import get_axon_ntff_profile_hook
        return get_axon_ntff_profile_hook() is not None
    except ImportError:
        pass
    try:
        import types
        import antenv
        from trn_agent_boot.trn_boot import _ntff_profile_via_ctypes
        mod = types.ModuleType("antenv.axon_hooks")
        mod._hook = _ntff_profile_via_ctypes("/opt/axon/libaxon_pjrt.so")
        mod.get_axon_ntff_profile_hook = lambda: mod._hook
        mod.set_axon_ntff_profile_hook = (
            lambda h: setattr(mod, "_hook", h))
        sys.modules["antenv.axon_hooks"] = mod
        antenv.axon_hooks = mod
        return mod._hook is not None
    except Exception:
        return False


def kernel(**inputs):
    global LAST_RESULTS
    in_maps, meta = _prep(inputs)
    if meta not in _BUILT:
        _BUILT[meta] = _build(meta)
    nc = _BUILT[meta]
    trace = bool(int(os.environ.get("BASS_TRACE", "0")))
    if trace:
        trace = _ensure_ntff_hook()
    if trace:
        # zero-egress container: keep profiling artifacts local
        bass_utils.upload_artifacts = lambda d: d
    try:
        res = bass_utils.run_bass_kernel_spmd(
            nc, in_maps, core_ids=list(range(NCORES)), trace=trace)
    except Exception:
        if not trace:
            raise
        os.environ["BASS_NEVER_TRACE"] = "1"
        res = bass_utils.run_bass_kernel_spmd(
            nc, in_maps, core_ids=list(range(NCORES)), trace=False)
    LAST_RESULTS = res
    out = np.empty((N_READ, 2), np.float32)
    for c in range(NCORES):
        out[c * R_LOC:(c + 1) * R_LOC] = res.results[c]["out"][:R_LOC]
    return out



# revision 6
# speedup vs baseline: 1.4703x; 1.4703x over previous
"""Trainium2 Bass kernel for nn_BipartiteGCN (6-layer bipartite GCN,
200K read nodes, 50K intron nodes, 2M random edges).

Strategy (8 NeuronCores, SPMD):
 - Shard edges by READ-node range: core c owns reads [25000c, 25000(c+1))
   and ALL edges incident to them.  Read-side aggregations are then exact
   and local; intron-side aggregations produce partials that are combined
   with ReduceScatter (+AllGather of the processed feature tables).
 - Per layer, node features live in DRAM tables of 256B bf16 rows
   (feature dim padded to 128).  The random side of each layer's
   gather/scatter is handled by batched SWDGE dma_gather (256B rows,
   int16 indices, tables kept < 32768 rows by splitting the intron table
   into two halves); the sorted side is handled by one-hot matmuls on the
   TensorEngine accumulating segment sums in PSUM.
 - One-hots are built on VectorE as bf16 is_equal against a pre-built
   replicated iota, 32 blocks (4096 edges) per instruction.
 - Degree vectors (pure functions of the integer edge lists, like the
   sort/padding metadata itself) are computed host-side as bincounts; the
   float math 1/sqrt(max(deg,1)) and everything downstream runs on device.
 - D^-1/2 scalings are folded into table construction (source side) and
   into per-partition activation scales at PSUM-drain time (dest side,
   using relu(s*x) = s*relu(x) for s>0).
"""

import os
import sys
import numpy as np
import ml_dtypes

sys.path.insert(0, "/opt/trn_rl_repo")

from contextlib import ExitStack

import concourse.bass as bass
import concourse.tile as tile
import concourse.mybir as mybir
from concourse import bacc, bass_utils
from concourse.masks import make_identity

P = 128
NCORES = 8
N_READ = 200000
N_INTRON = 50000
N_EDGES = 2000000

R_LOC = 25000          # real reads per core
R_PAD = 25088          # padded read slots per core (196 chunks)
NWIN = 196             # read windows (chunks of 128)
ISLOT = 50176          # padded intron slots (392 chunks)
NCH_A = 392            # intron chunks
IHALF = 25088          # intron slots per half
RSROWS = 25600         # RS payload rows per half (divisible: 3200/rank)
SHARD = 3200           # rows per rank after RS
NSC = 25               # shard chunks of 128 rows
SENT_A = R_PAD - 1     # sentinel gather row in TA tables (zero row)
SENT_B = IHALF         # sentinel gather row in TB tables (zero row)
GRP = 32               # blocks per one-hot instruction
GB = 32                # blocks per dma_gather call

bf16 = mybir.dt.bfloat16
f32 = mybir.dt.float32
i16 = mybir.dt.int16

_BUILT = {}
LAST_RESULTS = None


# ----------------------------------------------------------------------
# host-side prep
# ----------------------------------------------------------------------

def _wrap_idx(a):
    """[N] -> [128, N//16] int16 wrapped layout for dma_gather."""
    w = a.astype(np.int16).reshape(-1, 16).T
    return np.ascontiguousarray(np.tile(w, (8, 1)))


def _pmaj(a, nblk):
    """flat [nblk*128] -> [128, nblk] partition-major (e -> [e%128, e//128])"""
    return np.ascontiguousarray(a.reshape(nblk, P).T)


def _build_order(gid, payload_idx, payload_rel, n_groups, blk):
    """Scatter edges (sorted by group id) into padded per-group block slots.

    gid: [n] group id per edge;  blk: [n_groups] blocks per group (shared
    across cores).  Returns (gidx_pad int64, rel_pad int64) of length
    sum(blk)*128 with sentinel -1 in unfilled slots."""
    order = np.argsort(gid, kind="stable")
    gs = gid[order]
    cnt = np.bincount(gid, minlength=n_groups)
    raw_start = np.concatenate([[0], np.cumsum(cnt)[:-1]])
    pad_start = np.concatenate([[0], np.cumsum(blk)[:-1]]) * P
    n = gid.shape[0]
    pos_in_g = np.arange(n) - np.repeat(raw_start, cnt)
    pos = pad_start[gs] + pos_in_g
    tot = int(blk.sum()) * P
    gidx = np.full(tot, -1, np.int64)
    rel = np.zeros(tot, np.int64)
    gidx[pos] = payload_idx[order]
    rel[pos] = payload_rel[order]
    return gidx, rel


def _prep(inputs):
    src = np.asarray(inputs["edge_src"]).astype(np.int64)
    dst = np.asarray(inputs["edge_dst"]).astype(np.int64)
    h_read = np.asarray(inputs["h_read"]).astype(np.float32)

    core = src // R_LOC
    deg_i_glob = np.bincount(dst, minlength=ISLOT).astype(np.float32)

    per_core = []
    cntA = np.zeros((NCORES, NCH_A), np.int64)
    cntB = np.zeros((NCORES, 2 * NWIN), np.int64)
    for c in range(NCORES):
        m = core == c
        s = src[m] - c * R_LOC
        d = dst[m]
        chA = d // P                       # order-A group: intron chunk
        half = d // IHALF
        gB = half * NWIN + (s // P)        # order-B group: (half, window)
        cntA[c] = np.bincount(chA, minlength=NCH_A)
        cntB[c] = np.bincount(gB, minlength=2 * NWIN)
        per_core.append((s, d, chA, gB, half))

    blkA = np.maximum(1, -(-cntA.max(axis=0) // P))
    blkB = np.maximum(1, -(-cntB.max(axis=0) // P))
    nblkA = int(blkA.sum())
    nblkB = int(blkB.sum())

    in_maps = []
    for c in range(NCORES):
        s, d, chA, gB, half = per_core[c]
        gA, relA = _build_order(chA, s, d % P, NCH_A, blkA)
        gA[gA < 0] = SENT_A
        gBi, relB = _build_order(gB, d - half * IHALF, s % P, 2 * NWIN, blkB)
        gBi[gBi < 0] = SENT_B

        deg_r = np.bincount(s, minlength=R_PAD).astype(np.float32)
        # my shard slice of intron degrees: col j = h*NSC + cc;
        # slot = h*IHALF + c*SHARD + cc*128 + p  (junk rows -> 1.0)
        deg_my = np.ones((P, 2 * NSC), np.float32)
        for h in range(2):
            rows = c * SHARD + np.arange(NSC * P)
            valid = rows < IHALF
            slots = h * IHALF + rows
            v = np.ones(NSC * P, np.float32)
            v[valid] = deg_i_glob[slots[valid]]
            deg_my[:, h * NSC:(h + 1) * NSC] = v.reshape(NSC, P).T

        hrT = np.zeros((10, R_PAD), np.float32)
        hrT[:, :R_LOC] = h_read[c * R_LOC:(c + 1) * R_LOC].T

        bf = ml_dtypes.bfloat16
        im = {
            "gidxA": _wrap_idx(gA),
            "drelA": _pmaj(relA, nblkA).astype(bf),
            "gidxB": _wrap_idx(gBi),
            "drelB": _pmaj(relB, nblkB).astype(bf),
            "hrT": hrT,
            "degr": np.ascontiguousarray(
                deg_r.reshape(NWIN, P).T).astype(np.float32),
            "degi": deg_my,
            "w0": np.asarray(inputs["W0"]).astype(np.float32),
            "wext1": np.concatenate(
                [np.asarray(inputs["W1"]),
                 np.asarray(inputs["b1"])[None, :]], 0).astype(bf),
            "atts": np.asarray(inputs["atts"]).reshape(1, 6)
                      .astype(np.float32),
            "fcw": np.asarray(inputs["fc_w"]).astype(bf),
            "fcb": np.asarray(inputs["fc_b"]).reshape(1, 2).astype(bf),
        }
        for l in (2, 3, 4, 5):
            im[f"w{l}"] = np.asarray(inputs[f"W{l}"]).astype(bf)
            im[f"b{l}"] = np.asarray(inputs[f"b{l}"]).reshape(1, P).astype(bf)
        in_maps.append(im)

    meta = (tuple(int(x) for x in blkA), tuple(int(x) for x in blkB))
    return in_maps, meta


# ----------------------------------------------------------------------
# device program
# ----------------------------------------------------------------------

def _build(meta):
    blkA, blkB = (np.array(meta[0]), np.array(meta[1]))
    nblkA, nblkB = int(blkA.sum()), int(blkB.sum())

    nc = bacc.Bacc("TRN2", target_bir_lowering=False, debug=False,
                   num_devices=NCORES, num_swdge_queues=4)

    # --- I/O ---
    t_gidxA = nc.dram_tensor("gidxA", [P, nblkA * 8], i16, kind="ExternalInput")
    t_drelA = nc.dram_tensor("drelA", [P, nblkA], bf16, kind="ExternalInput")
    t_gidxB = nc.dram_tensor("gidxB", [P, nblkB * 8], i16, kind="ExternalInput")
    t_drelB = nc.dram_tensor("drelB", [P, nblkB], bf16, kind="ExternalInput")
    t_hrT = nc.dram_tensor("hrT", [10, R_PAD], f32, kind="ExternalInput")
    t_degr = nc.dram_tensor("degr", [P, NWIN], f32, kind="ExternalInput")
    t_degi = nc.dram_tensor("degi", [P, 2 * NSC], f32, kind="ExternalInput")
    t_w0 = nc.dram_tensor("w0", [10, 64], f32, kind="ExternalInput")
    t_wext1 = nc.dram_tensor("wext1", [65, P], bf16, kind="ExternalInput")
    t_w = {l: nc.dram_tensor(f"w{l}", [P, P], bf16, kind="ExternalInput")
           for l in (2, 3, 4, 5)}
    t_b = {l: nc.dram_tensor(f"b{l}", [1, P], bf16, kind="ExternalInput")
           for l in (2, 3, 4, 5)}
    t_fcw = nc.dram_tensor("fcw", [P, 2], bf16, kind="ExternalInput")
    t_fcb = nc.dram_tensor("fcb", [1, 2], bf16, kind="ExternalInput")
    t_atts = nc.dram_tensor("atts", [1, 6], f32, kind="ExternalInput")
    t_out = nc.dram_tensor("out", [R_PAD, 2], f32, kind="ExternalOutput")

    groups = [list(range(NCORES))]

    with tile.TileContext(nc) as tc, ExitStack() as ctx:
        cst = ctx.enter_context(tc.tile_pool(name="cst", bufs=1))
        sb = ctx.enter_context(tc.tile_pool(name="sb", bufs=2))
        gpool = ctx.enter_context(tc.tile_pool(name="gp", bufs=6))
        ohpool = ctx.enter_context(tc.tile_pool(name="oh", bufs=2))
        ps1p = ctx.enter_context(tc.tile_pool(name="ps1", bufs=3, space="PSUM"))
        ps2p = ctx.enter_context(tc.tile_pool(name="ps2", bufs=2, space="PSUM"))
        ps3p = ctx.enter_context(tc.tile_pool(name="ps3", bufs=2, space="PSUM"))
        dram = ctx.enter_context(tc.tile_pool(name="dr", bufs=1, space="DRAM"))

        # ---------- constants ----------
        iota_i = cst.tile([P, P], mybir.dt.int32)
        nc.gpsimd.iota(iota_i[:], pattern=[[1, P]], base=0,
                       channel_multiplier=0)
        iota_bf = cst.tile([P, P], bf16)
        nc.vector.tensor_copy(iota_bf[:], iota_i[:])
        iota_rep = cst.tile([P, P, GRP], bf16)
        nc.vector.tensor_copy(
            iota_rep[:], iota_bf[:].unsqueeze(2).to_broadcast([P, P, GRP]))

        ident_f = cst.tile([P, P], f32)
        make_identity(nc, ident_f[:])
        ident_b = cst.tile([P, P], bf16)
        nc.vector.tensor_copy(ident_b[:], ident_f[:])

        ones_f = cst.tile([1, P], f32)
        nc.vector.memset(ones_f[:], 1.0)
        ones_b = cst.tile([1, P], bf16)
        nc.vector.memset(ones_b[:], 1.0)

        zero_sb = cst.tile([P, 4, P], bf16)
        nc.vector.memset(zero_sb[:], 0.0)

        # weights
        w0_sb = cst.tile([10, 64], f32)
        nc.sync.dma_start(w0_sb[:], t_w0[:])
        wext1_sb = cst.tile([65, P], bf16)
        nc.sync.dma_start(wext1_sb[:], t_wext1[:])
        w_sb, b_sb = {}, {}
        for l in (2, 3, 4, 5):
            w_sb[l] = cst.tile([P, P], bf16, tag=f"w{l}", name=f"w{l}sb")
            nc.sync.dma_start(w_sb[l][:], t_w[l][:])
            b_sb[l] = cst.tile([1, P], bf16, tag=f"b{l}", name=f"b{l}sb")
            nc.sync.dma_start(b_sb[l][:], t_b[l][:])
        fcw_sb = cst.tile([P, 2], bf16)
        nc.sync.dma_start(fcw_sb[:], t_fcw[:])
        fcb_sb = cst.tile([1, 2], bf16)
        nc.sync.dma_start(fcb_sb[:], t_fcb[:])

        # gates: sigmoid(atts) replicated to 128 partitions
        atts_sb = cst.tile([1, 6], f32)
        nc.sync.dma_start(atts_sb[:], t_atts[:])
        sg = cst.tile([1, 6], f32)
        nc.scalar.activation(sg[:], atts_sb[:],
                             mybir.ActivationFunctionType.Sigmoid)
        ps_g = ps3p.tile([P, 6], f32, tag="p3")
        nc.tensor.matmul(ps_g[:], lhsT=ones_f[:], rhs=sg[:],
                         start=True, stop=True)
        g_rep = cst.tile([P, 6], f32)
        nc.scalar.copy(g_rep[:], ps_g[:])

        # rs_r = 1/sqrt(max(deg_r,1)); per-layer drain scales
        degr_sb = cst.tile([P, NWIN], f32)
        nc.sync.dma_start(degr_sb[:], t_degr[:])
        rs_r = cst.tile([P, NWIN], f32)
        nc.vector.tensor_scalar_max(rs_r[:], degr_sb[:], 1.0)
        nc.scalar.sqrt(rs_r[:], rs_r[:])
        nc.vector.reciprocal(rs_r[:], rs_r[:])
        rgr = {}
        for l in (1, 3, 5):
            rgr[l] = cst.tile([P, NWIN], f32, tag=f"rgr{l}", name=f"rgr{l}")
            if l == 5:
                # last conv layer: no outer rs_r fold -> scale = rs_r * g5
                nc.vector.tensor_copy(rgr[l][:], rs_r[:])
            else:
                nc.vector.tensor_tensor(out=rgr[l][:], in0=rs_r[:],
                                        in1=rs_r[:], op=mybir.AluOpType.mult)
            nc.vector.tensor_tensor(
                out=rgr[l][:], in0=rgr[l][:],
                in1=g_rep[:, l:l + 1].to_broadcast([P, NWIN]),
                op=mybir.AluOpType.mult)

        degi_sb = cst.tile([P, 2 * NSC], f32)
        nc.sync.dma_start(degi_sb[:], t_degi[:])
        rs_i = cst.tile([P, 2 * NSC], f32)
        nc.vector.tensor_scalar_max(rs_i[:], degi_sb[:], 1.0)
        nc.scalar.sqrt(rs_i[:], rs_i[:])
        nc.vector.reciprocal(rs_i[:], rs_i[:])
        r2g = {}
        for l in (0, 2, 4):
            r2g[l] = cst.tile([P, 2 * NSC], f32, tag=f"r2g{l}", name=f"r2g{l}")
            nc.vector.tensor_tensor(out=r2g[l][:], in0=rs_i[:], in1=rs_i[:],
                                    op=mybir.AluOpType.mult)
            nc.vector.tensor_tensor(
                out=r2g[l][:], in0=r2g[l][:],
                in1=g_rep[:, l:l + 1].to_broadcast([P, 2 * NSC]),
                op=mybir.AluOpType.mult)

        # uT staging buffers with a fixed ones row (fin=64 path)
        uT65 = [cst.tile([65, P], bf16, tag=f"uT65_{i}", name=f"uT65_{i}")
                for i in range(2)]
        for t in uT65:
            nc.vector.memset(t[:], 1.0)
        uT128 = [cst.tile([P, P], bf16, tag=f"uT128_{i}", name=f"uT128_{i}")
                 for i in range(2)]

        # big shared buffer: z0 staging / order-B aggregation
        bigbuf = cst.tile([P, NWIN, P], bf16, tag="bigbuf")

        # DRAM tables & collective buffers
        TA = [dram.tile([R_PAD, P], bf16, tag=f"TA{k}", name=f"TA{k}")
              for k in range(3)]
        TB = {}
        rsin, rsout, zsh = {}, {}, {}
        for l in (0, 2, 4):
            TB[l] = [dram.tile([RSROWS, P], bf16, tag=f"TB{l}_{h}",
                               name=f"TB{l}_{h}",
                               addr_space="Shared") for h in range(2)]
            rsin[l] = [dram.tile([RSROWS, P], bf16, tag=f"rsin{l}_{h}",
                                 name=f"rsin{l}_{h}")
                       for h in range(2)]
            rsout[l] = [dram.tile([SHARD, P], bf16, tag=f"rso{l}_{h}",
                                  name=f"rso{l}_{h}")
                        for h in range(2)]
            zsh[l] = [dram.tile([SHARD, P], bf16, tag=f"zsh{l}_{h}",
                                name=f"zsh{l}_{h}")
                      for h in range(2)]
            for h in range(2):
                nc.sync.dma_start(
                    rsin[l][h][IHALF:RSROWS, :]
                    .rearrange("(c p) f -> p c f", p=P),
                    zero_sb[:])

        # ---------- z0 = (h_read * rs_r) @ W0  ->  TA[0] ----------
        nc.vector.memset(bigbuf[:], 0.0)
        PIECE = 16
        for p0 in range(0, NWIN, PIECE):
            pw = min(PIECE, NWIN - p0)
            hrp = sb.tile([10, pw * P], f32, tag="hrp")
            nc.sync.dma_start(hrp[:], t_hrT[:, p0 * P:(p0 + pw) * P])
            for wl in range(pw):
                w = p0 + wl
                psz = ps3p.tile([P, 64], f32, tag="p3")
                nc.tensor.matmul(psz[:], lhsT=hrp[:, wl * P:(wl + 1) * P],
                                 rhs=w0_sb[:], start=True, stop=True)
                nc.scalar.mul(bigbuf[:, w, :64], psz[:], rs_r[:, w:w + 1])
        nc.sync.dma_start(
            TA[0][:].rearrange("(w p) f -> p w f", p=P), bigbuf[:])

        # ---------- pass machinery ----------
        gq_counter = [0]  # global SWDGE-DMA issue counter (queue rotation)

        def gather_plan(blk, half_split_blocks):
            """split blocks into dma_gather calls of <=GB blocks, not
            crossing the half boundary (in block index space)."""
            calls = []
            for lo, hi in half_split_blocks:
                b = lo
                while b < hi:
                    n = min(GB, hi - b)
                    calls.append((b, n))
                    b += n
            return calls

        def scatter_pass(idx_dram, drel_dram, nblk, blk, tables, on_group):
            """Generic pass: gather + one-hot + psum accumulate per group.

            tables: list of (in_ap, blocks_lo, blocks_hi) gather sources.
            on_group(g, ps1): consume the accumulated psum for group g.
            """
            idx_sb = cst.tile([P, max(nblkA, nblkB) * 8], i16, tag="idxsb")
            nc.sync.dma_start(idx_sb[:, :nblk * 8], idx_dram[:])
            drel_sb = cst.tile([P, max(nblkA, nblkB)], bf16, tag="drelsb")
            nc.sync.dma_start(drel_sb[:, :nblk], drel_dram[:])

            calls = gather_plan(blk, [(lo, hi) for _, lo, hi in tables])
            tbl_of_call = {}
            for ci, (b0, nb) in enumerate(calls):
                for ap, lo, hi in tables:
                    if lo <= b0 < hi:
                        tbl_of_call[ci] = ap
            call_of_block = {}
            for ci, (b0, nb) in enumerate(calls):
                for b in range(b0, b0 + nb):
                    call_of_block[b] = ci

            gtiles, ohtiles = {}, {}
            starts = np.concatenate([[0], np.cumsum(blk)]).astype(int)
            ngrp = len(blk)
            for g in range(ngrp):
                ps1 = ps1p.tile([P, P], f32, tag="p1")
                for b in range(starts[g], starts[g + 1]):
                    ci = call_of_block[b]
                    if ci not in gtiles:
                        b0, nb = calls[ci]
                        gt = gpool.tile([P, nb, P], bf16, tag="gbuf")
                        # Round-robin the 4 SWDGE queues so descriptor
                        # generation parallelizes across Q7 core pairs.
                        # queue = counter%4 with Tile's DMASW lane =
                        # counter%8 keeps lane-sharing gathers on one
                        # queue (FIFO), so lane sems stay ordered.
                        nc.gpsimd.dma_gather(
                            gt[:], tbl_of_call[ci], idx_sb[:, b0 * 8:
                                                          (b0 + nb) * 8],
                            nb * P, nb * P, P, single_packet=False,
                            queue_num=gq_counter[0] % 4)
                        gq_counter[0] += 1
                        gtiles[ci] = gt
                        # drop old refs so pool slots rotate
                        for k in list(gtiles):
                            if k < ci - 5:
                                del gtiles[k]
                    oi = b // GRP
                    if oi not in ohtiles:
                        o0 = oi * GRP
                        on = min(GRP, nblk - o0)
                        oh = ohpool.tile([P, P, on], bf16, tag="oh")
                        nc.vector.tensor_tensor(
                            out=oh[:],
                            in0=drel_sb[:, o0:o0 + on].unsqueeze(1)
                                .to_broadcast([P, P, on]),
                            in1=iota_rep[:, :, :on],
                            op=mybir.AluOpType.is_equal)
                        ohtiles[oi] = oh
                        for k in list(ohtiles):
                            if k < oi - 1:
                                del ohtiles[k]
                    b0, nb = calls[ci]
                    nc.tensor.matmul(
                        ps1[:],
                        lhsT=ohtiles[oi][:, :, b - oi * GRP],
                        rhs=gtiles[ci][:, b - b0, :],
                        start=(b == starts[g]),
                        stop=(b == starts[g + 1] - 1))
                on_group(g, ps1)

        # ---------- intron-side (order A) pass: l in {0,2,4} ----------
        def a_pass(l, ta_ap):
            astage = [None]

            def shard_and_ag(h):
                nc.gpsimd.collective_compute(
                    "ReduceScatter", mybir.AluOpType.add,
                    replica_groups=groups,
                    ins=[rsin[l][h][:].opt()], outs=[rsout[l][h][:].opt()])
                zstage = None
                for cc in range(NSC):
                    sh_in = sb.tile([P, P], bf16, tag="shin")
                    nc.sync.dma_start(
                        sh_in[:], rsout[l][h][cc * P:(cc + 1) * P, :])
                    col = h * NSC + cc
                    if cc % 4 == 0:
                        zstage = sb.tile([P, 4, P], bf16, tag="zst")
                    if l == 0:
                        nc.scalar.activation(
                            zstage[:, cc % 4, :], sh_in[:],
                            mybir.ActivationFunctionType.Relu,
                            scale=r2g[l][:, col:col + 1])
                    else:
                        ps2 = ps2p.tile([P, P], bf16, tag="p2")
                        nc.tensor.transpose(ps2[:], sh_in[:], ident_b[:])
                        uT = uT128[cc % 2]
                        nc.scalar.copy(uT[:], ps2[:])
                        ps3 = ps3p.tile([P, P], f32, tag="p3")
                        nc.tensor.matmul(ps3[:], lhsT=uT[:], rhs=w_sb[l][:],
                                         start=True, stop=False)
                        nc.tensor.matmul(ps3[:], lhsT=ones_b[:],
                                         rhs=b_sb[l][:],
                                         start=False, stop=True)
                        nc.scalar.activation(
                            zstage[:, cc % 4, :], ps3[:],
                            mybir.ActivationFunctionType.Relu,
                            scale=r2g[l][:, col:col + 1])
                    if cc % 4 == 3 or cc == NSC - 1:
                        c0 = cc - cc % 4
                        nc.sync.dma_start(
                            zsh[l][h][c0 * P:(cc + 1) * P, :]
                            .rearrange("(c p) f -> p c f", p=P),
                            zstage[:, :cc % 4 + 1, :])
                # NOTE: TB tail rows (>= IHALF, incl. the sentinel row) are
                # zero because the rsin tail is zeroed and biases are zero.
                nc.gpsimd.collective_compute(
                    "AllGather", mybir.AluOpType.bypass,
                    replica_groups=groups,
                    ins=[zsh[l][h][:].opt()], outs=[TB[l][h][:].opt()])

            def on_group(ch, ps1):
                if ch % 4 == 0:
                    astage[0] = sb.tile([P, 4, P], bf16, tag="ast", name="ast")
                nc.scalar.copy(astage[0][:, ch % 4, :], ps1[:])
                if ch % 4 == 3:
                    h, chl = ch // NWIN, (ch - ch // NWIN * NWIN)
                    c0 = chl - 3
                    nc.sync.dma_start(
                        rsin[l][h][c0 * P:(chl + 1) * P, :]
                        .rearrange("(c p) f -> p c f", p=P), astage[0][:])

            scatter_pass(t_gidxA[:], t_drelA[:], nblkA, blkA,
                         [(ta_ap, 0, nblkA)], on_group)
            # SWDGE gathers in flight concurrently with ncfw collectives
            # deadlock on this stack -- hard-serialize the pass tail.
            tc.strict_bb_all_engine_barrier()
            shard_and_ag(0)
            shard_and_ag(1)
            tc.strict_bb_all_engine_barrier()

        # ---------- read-side (order B) pass: l in {1,3,5} ----------
        bstartsB = np.concatenate([[0], np.cumsum(blkB)]).astype(int)
        half_split_B = int(bstartsB[NWIN])

        def b_pass(l, out_sink):
            def post_window(w):
                fin = 64 if l == 1 else P
                ps2 = ps2p.tile([fin, P], bf16, tag="p2")
                nc.tensor.transpose(ps2[:], bigbuf[:, w, :fin], ident_b[:])
                if l == 1:
                    uT = uT65[w % 2]
                    nc.scalar.copy(uT[:64, :], ps2[:])
                    ps3 = ps3p.tile([P, P], f32, tag="p3")
                    nc.tensor.matmul(ps3[:], lhsT=uT[:], rhs=wext1_sb[:],
                                     start=True, stop=True)
                else:
                    uT = uT128[w % 2]
                    nc.scalar.copy(uT[:], ps2[:])
                    ps3 = ps3p.tile([P, P], f32, tag="p3")
                    nc.tensor.matmul(ps3[:], lhsT=uT[:], rhs=w_sb[l][:],
                                     start=True, stop=False)
                    nc.tensor.matmul(ps3[:], lhsT=ones_b[:], rhs=b_sb[l][:],
                                     start=False, stop=True)
                out_sink(w, ps3)

            def on_group(g, ps1):
                h, w = g // NWIN, g % NWIN
                if h == 0:
                    nc.scalar.copy(bigbuf[:, w, :], ps1[:])
                else:
                    nc.vector.tensor_tensor(
                        out=bigbuf[:, w, :], in0=ps1[:], in1=bigbuf[:, w, :],
                        op=mybir.AluOpType.add)
                    post_window(w)

            tabs = [(TB[l - 1][0][:], 0, half_split_B),
                    (TB[l - 1][1][:], half_split_B, nblkB)]
            scatter_pass(t_gidxB[:], t_drelB[:], nblkB, blkB, tabs, on_group)

        # L0
        a_pass(0, TA[0][:])

        # L1 -> TA[1]
        zr_stage = [None]

        def sink_l1(w, ps3, l=1, k=1):
            if w % 4 == 0:
                zr_stage[0] = sb.tile([P, 4, P], bf16, tag="zrst", name="zrst")
            nc.scalar.activation(zr_stage[0][:, w % 4, :], ps3[:],
                                 mybir.ActivationFunctionType.Relu,
                                 scale=rgr[l][:, w:w + 1])
            if w % 4 == 3:
                c0 = w - 3
                nc.sync.dma_start(
                    TA[k][c0 * P:(w + 1) * P, :]
                    .rearrange("(c p) f -> p c f", p=P), zr_stage[0][:])

        b_pass(1, sink_l1)
        a_pass(2, TA[1][:])
        b_pass(3, lambda w, ps3: sink_l1(w, ps3, l=3, k=2))
        a_pass(4, TA[2][:])

        # L5 -> fc -> out
        out_sb = cst.tile([P, NWIN, 2], f32, tag="outsb")

        def sink_l5(w, ps3):
            h5 = sb.tile([P, P], f32, tag="h5")
            nc.scalar.activation(h5[:], ps3[:],
                                 mybir.ActivationFunctionType.Relu,
                                 scale=rgr[5][:, w:w + 1])
            ps2b = ps2p.tile([P, P], f32, tag="p2")
            nc.tensor.transpose(ps2b[:], h5[:], ident_f[:])
            h5T = uT128[w % 2]
            nc.scalar.copy(h5T[:], ps2b[:])
            psf = ps3p.tile([P, 2], f32, tag="p3")
            nc.tensor.matmul(psf[:], lhsT=h5T[:], rhs=fcw_sb[:],
                             start=True, stop=False)
            nc.tensor.matmul(psf[:], lhsT=ones_b[:], rhs=fcb_sb[:],
                             start=False, stop=True)
            nc.scalar.copy(out_sb[:, w, :], psf[:])

        b_pass(5, sink_l5)
        nc.sync.dma_start(
            t_out[:].rearrange("(w p) c -> p w c", p=P), out_sb[:])

    nc.compile()
    return nc


# ----------------------------------------------------------------------
# entry point
# ----------------------------------------------------------------------

def _ensure_ntff_hook():
    """Install the axon NTFF profiling hook shim if the image's antenv
    lacks the axon_hooks module (profiling-only; safe to fail)."""
    try:
        from antenv.axon_hooks import get_axon_ntff_profile_hook
        return get_axon_ntff_profile_hook() is not None
    except ImportError:
        pass
    try:
        import types
        import antenv
        from trn_agent_boot.trn_boot import _ntff_profile_via_ctypes
        mod = types.ModuleType("antenv.axon_hooks")
        mod._hook = _ntff_profile_via_ctypes("/opt/axon/libaxon_pjrt.so")
        mod.get_axon_ntff_profile_hook = lambda: mod._hook
        mod.set_axon_ntff_profile_hook = (
            lambda h: setattr(mod, "_hook", h))
        sys.modules["antenv.axon_hooks"] = mod
        antenv.axon_hooks = mod
        return mod._hook is not None
    except Exception:
        return False


def kernel(**inputs):
    global LAST_RESULTS
    in_maps, meta = _prep(inputs)
    if meta not in _BUILT:
        _BUILT[meta] = _build(meta)
    nc = _BUILT[meta]
    trace = bool(int(os.environ.get("BASS_TRACE", "0")))
    if trace:
        trace = _ensure_ntff_hook()
    if trace:
        # zero-egress container: keep profiling artifacts local
        bass_utils.upload_artifacts = lambda d: d
    try:
        res = bass_utils.run_bass_kernel_spmd(
            nc, in_maps, core_ids=list(range(NCORES)), trace=trace)
    except Exception:
        if not trace:
            raise
        os.environ["BASS_NEVER_TRACE"] = "1"
        res = bass_utils.run_bass_kernel_spmd(
            nc, in_maps, core_ids=list(range(NCORES)), trace=False)
    LAST_RESULTS = res
    out = np.empty((N_READ, 2), np.float32)
    for c in range(NCORES):
        out[c * R_LOC:(c + 1) * R_LOC] = res.results[c]["out"][:R_LOC]
    return out



# revision 18
# speedup vs baseline: 3.1094x; 2.1148x over previous
"""Trainium2 Bass kernel for nn_BipartiteGCN (6-layer bipartite GCN,
200K read nodes, 50K intron nodes, 2M random edges).

Strategy (8 NeuronCores, SPMD):
 - Shard edges by READ-node range: core c owns reads [25000c, 25000(c+1))
   and ALL edges incident to them.  Read-side aggregations are then exact
   and local; intron-side aggregations produce partials that are combined
   with ReduceScatter (+AllGather of the processed feature tables).
 - Per layer, node features live in DRAM tables of 256B bf16 rows
   (feature dim padded to 128).  The random side of each layer's
   gather/scatter is handled by batched SWDGE dma_gather (256B rows,
   int16 indices, tables kept < 32768 rows by splitting the intron table
   into two halves); the sorted side is handled by one-hot matmuls on the
   TensorEngine accumulating segment sums in PSUM.
 - One-hots are built on VectorE as bf16 is_equal against a pre-built
   replicated iota, 32 blocks (4096 edges) per instruction.
 - Degree vectors (pure functions of the integer edge lists, like the
   sort/padding metadata itself) are computed host-side as bincounts; the
   float math 1/sqrt(max(deg,1)) and everything downstream runs on device.
 - D^-1/2 scalings are folded into table construction (source side) and
   into per-partition activation scales at PSUM-drain time (dest side,
   using relu(s*x) = s*relu(x) for s>0).
"""

import os
import sys
import numpy as np
import ml_dtypes

sys.path.insert(0, "/opt/trn_rl_repo")

from contextlib import ExitStack

import concourse.bass as bass
import concourse.tile as tile
import concourse.mybir as mybir
from concourse import bacc, bass_utils
from concourse.masks import make_identity

P = 128
NCORES = 8
N_READ = 200000
N_INTRON = 50000
N_EDGES = 2000000

R_LOC = 25000          # real reads per core
R_PAD = 25600          # padded read slots per core (200 windows)
NWIN = 200             # read windows (chunks of 128); == NCH_A//2
ISLOT = 51200          # padded intron slots (400 chunks)
NCH_A = 400            # intron chunks
IHALF = 25600          # intron slots per half (== RSROWS: no tail pad)
RSROWS = 25600         # RS payload rows per half (divisible: 3200/rank)
SHARD = 3200           # rows per rank after RS
NSC = 25               # shard chunks of 128 rows
SENT_A = R_PAD - 1     # sentinel gather row in TA tables (kept-empty slot)
SENT_B = IHALF - 1     # sentinel gather row in TB tables (kept-empty slot)
GRP = 32               # blocks per one-hot instruction
GB = 32                # blocks per dma_gather call

bf16 = mybir.dt.bfloat16
f32 = mybir.dt.float32
i16 = mybir.dt.int16

_BUILT = {}
LAST_RESULTS = None


# ----------------------------------------------------------------------
# host-side prep
# ----------------------------------------------------------------------

def _wrap_idx(a):
    """[N] -> [128, N//16] int16 wrapped layout for dma_gather."""
    w = a.astype(np.int16).reshape(-1, 16).T
    return np.ascontiguousarray(np.tile(w, (8, 1)))


def _pmaj(a, nblk):
    """flat [nblk*128] -> [128, nblk] partition-major (e -> [e%128, e//128])"""
    return np.ascontiguousarray(a.reshape(nblk, P).T)


def _build_order(gid, payload_idx, payload_rel, n_groups, blk):
    """Scatter edges (sorted by group id, then ascending gather idx for
    DRAM locality in the SDMA drain) into padded per-group block slots.

    gid: [n] group id per edge;  blk: [n_groups] blocks per group (shared
    across cores).  Returns (gidx_pad int64, rel_pad int64) of length
    sum(blk)*128 with sentinel -1 in unfilled slots."""
    order = np.lexsort((payload_idx, gid))
    gs = gid[order]
    cnt = np.bincount(gid, minlength=n_groups)
    raw_start = np.concatenate([[0], np.cumsum(cnt)[:-1]])
    pad_start = np.concatenate([[0], np.cumsum(blk)[:-1]]) * P
    n = gid.shape[0]
    pos_in_g = np.arange(n) - np.repeat(raw_start, cnt)
    pos = pad_start[gs] + pos_in_g
    tot = int(blk.sum()) * P
    gidx = np.full(tot, -1, np.int64)
    rel = np.zeros(tot, np.int64)
    gidx[pos] = payload_idx[order]
    rel[pos] = payload_rel[order]
    return gidx, rel


def _pack_bins(w, nbins, caps):
    """Greedy multi-dim bin packing: place items (rows of w, heaviest
    total first) into nbins bins minimizing the max per-dimension bin sum,
    respecting per-bin item capacity caps.  Returns bin id per item."""
    order = np.argsort(-w.sum(1), kind="stable")
    sums = np.zeros((nbins, w.shape[1]))
    cnt = np.zeros(nbins, np.int64)
    asg = np.empty(len(w), np.int64)
    for i in order:
        s = (sums + w[i]).max(1)
        s[cnt >= caps] = np.inf
        b = int(np.argmin(s))
        asg[i] = b
        sums[b] += w[i]
        cnt[b] += 1
    return asg


def _bins_to_slots(asg, nbins):
    """bin assignment -> slot id (bin*128 + position within bin)."""
    order = np.argsort(asg, kind="stable")
    sorted_bins = asg[order]
    start = np.searchsorted(sorted_bins, np.arange(nbins))
    pos = np.arange(len(asg)) - start[sorted_bins]
    slot = np.empty(len(asg), np.int64)
    slot[order] = sorted_bins * P + pos
    return slot


def _prep(inputs):
    src = np.asarray(inputs["edge_src"]).astype(np.int64)
    dst = np.asarray(inputs["edge_dst"]).astype(np.int64)
    h_read = np.asarray(inputs["h_read"]).astype(np.float32)

    core = src // R_LOC

    # --- balance introns into chunks (8-dim per-core degree balance) so
    # every (chunk, core) count fits 5 blocks of 128; keep the last slot
    # of each half empty (sentinel rows must stay zero).
    deg8 = np.zeros((N_INTRON, NCORES), np.int64)
    np.add.at(deg8, (dst, core), 1)
    capsA = np.full(NCH_A, P, np.int64)
    capsA[NCH_A // 2 - 1] = P - 1
    capsA[NCH_A - 1] = P - 1
    iperm = _bins_to_slots(_pack_bins(deg8.astype(np.float64), NCH_A, capsA),
                           NCH_A)
    d_slot_all = iperm[dst]
    deg_i_glob = np.bincount(d_slot_all, minlength=ISLOT).astype(np.float32)

    per_core = []
    rperms = []
    cntA = np.zeros((NCORES, NCH_A), np.int64)
    cntB = np.zeros((NCORES, 2 * NWIN), np.int64)
    for c in range(NCORES):
        m = core == c
        s_raw = src[m] - c * R_LOC
        d = d_slot_all[m]
        half = d // IHALF
        # balance this core's reads into windows (per-half degree balance);
        # keep the last slot (SENT_A) empty.
        w2 = np.zeros((R_LOC, 2), np.int64)
        np.add.at(w2, (s_raw, half), 1)
        capsB = np.full(NWIN, P, np.int64)
        capsB[NWIN - 1] = P - 1
        rperm = _bins_to_slots(_pack_bins(w2.astype(np.float64), NWIN, capsB),
                               NWIN)
        rperms.append(rperm)
        s = rperm[s_raw]
        chA = d // P                       # order-A group: intron chunk
        gB = half * NWIN + (s // P)        # order-B group: (half, window)
        cntA[c] = np.bincount(chA, minlength=NCH_A)
        cntB[c] = np.bincount(gB, minlength=2 * NWIN)
        per_core.append((s, d, chA, gB, half))

    blkA = np.maximum(1, -(-cntA.max(axis=0) // P))
    blkB = np.maximum(1, -(-cntB.max(axis=0) // P))
    nblkA = int(blkA.sum())
    nblkB = int(blkB.sum())

    in_maps = []
    for c in range(NCORES):
        s, d, chA, gB, half = per_core[c]
        gA, relA = _build_order(chA, s, d % P, NCH_A, blkA)
        gA[gA < 0] = SENT_A
        gBi, relB = _build_order(gB, d - half * IHALF, s % P, 2 * NWIN, blkB)
        gBi[gBi < 0] = SENT_B

        deg_r = np.bincount(s, minlength=R_PAD).astype(np.float32)
        # my shard slice of intron degrees: col j = h*NSC + cc;
        # slot = h*IHALF + c*SHARD + cc*128 + p  (junk rows -> 1.0)
        deg_my = np.ones((P, 2 * NSC), np.float32)
        for h in range(2):
            rows = c * SHARD + np.arange(NSC * P)
            valid = rows < IHALF
            slots = h * IHALF + rows
            v = np.ones(NSC * P, np.float32)
            v[valid] = deg_i_glob[slots[valid]]
            deg_my[:, h * NSC:(h + 1) * NSC] = v.reshape(NSC, P).T

        hrT = np.zeros((10, R_PAD), np.float32)
        hrT[:, rperms[c]] = h_read[c * R_LOC:(c + 1) * R_LOC].T

        bf = ml_dtypes.bfloat16
        im = {
            "gidxA": _wrap_idx(gA),
            "drelA": _pmaj(relA, nblkA).astype(bf),
            "gidxB": _wrap_idx(gBi),
            "drelB": _pmaj(relB, nblkB).astype(bf),
            "hrT": hrT,
            "degr": np.ascontiguousarray(
                deg_r.reshape(NWIN, P).T).astype(np.float32),
            "degi": deg_my,
            "w0": np.asarray(inputs["W0"]).astype(np.float32),
            "wext1": np.concatenate(
                [np.asarray(inputs["W1"]),
                 np.asarray(inputs["b1"])[None, :]], 0).astype(bf),
            "atts": np.asarray(inputs["atts"]).reshape(1, 6)
                      .astype(np.float32),
            "fcw": np.asarray(inputs["fc_w"]).astype(bf),
            "fcb": np.asarray(inputs["fc_b"]).reshape(1, 2).astype(bf),
        }
        for l in (2, 3, 4, 5):
            im[f"w{l}"] = np.asarray(inputs[f"W{l}"]).astype(bf)
            im[f"b{l}"] = np.asarray(inputs[f"b{l}"]).reshape(1, P).astype(bf)
        in_maps.append(im)

    meta = (tuple(int(x) for x in blkA), tuple(int(x) for x in blkB))
    return in_maps, meta, rperms


# ----------------------------------------------------------------------
# device program
# ----------------------------------------------------------------------

def _build(meta):
    blkA, blkB = (np.array(meta[0]), np.array(meta[1]))
    nblkA, nblkB = int(blkA.sum()), int(blkB.sum())

    nc = bacc.Bacc("TRN2", target_bir_lowering=False, debug=False,
                   num_devices=NCORES, num_swdge_queues=4)

    # --- I/O ---
    t_gidxA = nc.dram_tensor("gidxA", [P, nblkA * 8], i16, kind="ExternalInput")
    t_drelA = nc.dram_tensor("drelA", [P, nblkA], bf16, kind="ExternalInput")
    t_gidxB = nc.dram_tensor("gidxB", [P, nblkB * 8], i16, kind="ExternalInput")
    t_drelB = nc.dram_tensor("drelB", [P, nblkB], bf16, kind="ExternalInput")
    t_hrT = nc.dram_tensor("hrT", [10, R_PAD], f32, kind="ExternalInput")
    t_degr = nc.dram_tensor("degr", [P, NWIN], f32, kind="ExternalInput")
    t_degi = nc.dram_tensor("degi", [P, 2 * NSC], f32, kind="ExternalInput")
    t_w0 = nc.dram_tensor("w0", [10, 64], f32, kind="ExternalInput")
    t_wext1 = nc.dram_tensor("wext1", [65, P], bf16, kind="ExternalInput")
    t_w = {l: nc.dram_tensor(f"w{l}", [P, P], bf16, kind="ExternalInput")
           for l in (2, 3, 4, 5)}
    t_b = {l: nc.dram_tensor(f"b{l}", [1, P], bf16, kind="ExternalInput")
           for l in (2, 3, 4, 5)}
    t_fcw = nc.dram_tensor("fcw", [P, 2], bf16, kind="ExternalInput")
    t_fcb = nc.dram_tensor("fcb", [1, 2], bf16, kind="ExternalInput")
    t_atts = nc.dram_tensor("atts", [1, 6], f32, kind="ExternalInput")
    t_out = nc.dram_tensor("out", [R_PAD, 2], f32, kind="ExternalOutput")

    groups = [list(range(NCORES))]

    with tile.TileContext(nc) as tc, ExitStack() as ctx:
        cst = ctx.enter_context(tc.tile_pool(name="cst", bufs=1))
        sb = ctx.enter_context(tc.tile_pool(name="sb", bufs=2))
        gpool = ctx.enter_context(tc.tile_pool(name="gp", bufs=6))
        ohpool = ctx.enter_context(tc.tile_pool(name="oh", bufs=2))
        ps1p = ctx.enter_context(tc.tile_pool(name="ps1", bufs=3, space="PSUM"))
        ps2p = ctx.enter_context(tc.tile_pool(name="ps2", bufs=2, space="PSUM"))
        ps3p = ctx.enter_context(tc.tile_pool(name="ps3", bufs=2, space="PSUM"))
        dram = ctx.enter_context(tc.tile_pool(name="dr", bufs=1, space="DRAM"))

        # ---------- constants ----------
        iota_i = cst.tile([P, P], mybir.dt.int32)
        nc.gpsimd.iota(iota_i[:], pattern=[[1, P]], base=0,
                       channel_multiplier=0)
        iota_bf = cst.tile([P, P], bf16)
        nc.vector.tensor_copy(iota_bf[:], iota_i[:])
        iota_rep = cst.tile([P, P, GRP], bf16)
        nc.vector.tensor_copy(
            iota_rep[:], iota_bf[:].unsqueeze(2).to_broadcast([P, P, GRP]))

        ident_f = cst.tile([P, P], f32)
        make_identity(nc, ident_f[:])
        ident_b = cst.tile([P, P], bf16)
        nc.vector.tensor_copy(ident_b[:], ident_f[:])

        ones_f = cst.tile([1, P], f32)
        nc.vector.memset(ones_f[:], 1.0)
        ones_b = cst.tile([1, P], bf16)
        nc.vector.memset(ones_b[:], 1.0)

        # weights
        w0_sb = cst.tile([10, 64], f32)
        nc.sync.dma_start(w0_sb[:], t_w0[:])
        wext1_sb = cst.tile([65, P], bf16)
        nc.sync.dma_start(wext1_sb[:], t_wext1[:])
        w_sb, b_sb = {}, {}
        for l in (2, 3, 4, 5):
            w_sb[l] = cst.tile([P, P], bf16, tag=f"w{l}", name=f"w{l}sb")
            nc.sync.dma_start(w_sb[l][:], t_w[l][:])
            b_sb[l] = cst.tile([1, P], bf16, tag=f"b{l}", name=f"b{l}sb")
            nc.sync.dma_start(b_sb[l][:], t_b[l][:])
        fcw_sb = cst.tile([P, 2], bf16)
        nc.sync.dma_start(fcw_sb[:], t_fcw[:])
        fcb_sb = cst.tile([1, 2], bf16)
        nc.sync.dma_start(fcb_sb[:], t_fcb[:])

        # gates: sigmoid(atts) replicated to 128 partitions
        atts_sb = cst.tile([1, 6], f32)
        nc.sync.dma_start(atts_sb[:], t_atts[:])
        sg = cst.tile([1, 6], f32)
        nc.scalar.activation(sg[:], atts_sb[:],
                             mybir.ActivationFunctionType.Sigmoid)
        ps_g = ps3p.tile([P, 6], f32, tag="p3")
        nc.tensor.matmul(ps_g[:], lhsT=ones_f[:], rhs=sg[:],
                         start=True, stop=True)
        g_rep = cst.tile([P, 6], f32)
        nc.scalar.copy(g_rep[:], ps_g[:])

        # rs_r = 1/sqrt(max(deg_r,1)); per-layer drain scales
        degr_sb = cst.tile([P, NWIN], f32)
        nc.sync.dma_start(degr_sb[:], t_degr[:])
        rs_r = cst.tile([P, NWIN], f32)
        nc.vector.tensor_scalar_max(rs_r[:], degr_sb[:], 1.0)
        nc.scalar.sqrt(rs_r[:], rs_r[:])
        nc.vector.reciprocal(rs_r[:], rs_r[:])
        rgr = {}
        for l in (1, 3, 5):
            rgr[l] = cst.tile([P, NWIN], f32, tag=f"rgr{l}", name=f"rgr{l}")
            if l == 5:
                # last conv layer: no outer rs_r fold -> scale = rs_r * g5
                nc.vector.tensor_copy(rgr[l][:], rs_r[:])
            else:
                nc.vector.tensor_tensor(out=rgr[l][:], in0=rs_r[:],
                                        in1=rs_r[:], op=mybir.AluOpType.mult)
            nc.vector.tensor_tensor(
                out=rgr[l][:], in0=rgr[l][:],
                in1=g_rep[:, l:l + 1].to_broadcast([P, NWIN]),
                op=mybir.AluOpType.mult)

        degi_sb = cst.tile([P, 2 * NSC], f32)
        nc.sync.dma_start(degi_sb[:], t_degi[:])
        rs_i = cst.tile([P, 2 * NSC], f32)
        nc.vector.tensor_scalar_max(rs_i[:], degi_sb[:], 1.0)
        nc.scalar.sqrt(rs_i[:], rs_i[:])
        nc.vector.reciprocal(rs_i[:], rs_i[:])
        r2g = {}
        for l in (0, 2, 4):
            r2g[l] = cst.tile([P, 2 * NSC], f32, tag=f"r2g{l}", name=f"r2g{l}")
            nc.vector.tensor_tensor(out=r2g[l][:], in0=rs_i[:], in1=rs_i[:],
                                    op=mybir.AluOpType.mult)
            nc.vector.tensor_tensor(
                out=r2g[l][:], in0=r2g[l][:],
                in1=g_rep[:, l:l + 1].to_broadcast([P, 2 * NSC]),
                op=mybir.AluOpType.mult)

        # uT staging buffers with a fixed ones row (fin=64 path)
        uT65 = [cst.tile([65, P], bf16, tag=f"uT65_{i}", name=f"uT65_{i}")
                for i in range(2)]
        for t in uT65:
            nc.vector.memset(t[:], 1.0)
        uT128 = [cst.tile([P, P], bf16, tag=f"uT128_{i}", name=f"uT128_{i}")
                 for i in range(2)]

        # big shared buffer: z0 staging / order-B aggregation
        bigbuf = cst.tile([P, NWIN, P], bf16, tag="bigbuf")

        # DRAM tables & collective buffers
        TA = [dram.tile([R_PAD, P], bf16, tag=f"TA{k}", name=f"TA{k}")
              for k in range(3)]
        TB = {}
        rsin, rsout, zsh = {}, {}, {}
        AW = {0: 64, 2: P, 4: P}   # aggregation width per a-layer
        for l in (0, 2, 4):
            TB[l] = [dram.tile([RSROWS, P], bf16, tag=f"TB{l}_{h}",
                               name=f"TB{l}_{h}",
                               addr_space="Shared") for h in range(2)]
            rsin[l] = [dram.tile([RSROWS, AW[l]], bf16, tag=f"rsin{l}_{h}",
                                 name=f"rsin{l}_{h}")
                       for h in range(2)]
            rsout[l] = [dram.tile([SHARD, AW[l]], bf16, tag=f"rso{l}_{h}",
                                  name=f"rso{l}_{h}")
                        for h in range(2)]
            zsh[l] = [dram.tile([SHARD, P], bf16, tag=f"zsh{l}_{h}",
                                name=f"zsh{l}_{h}")
                      for h in range(2)]

        # ---------- z0 = (h_read * rs_r) @ W0  ->  TA[0] ----------
        nc.vector.memset(bigbuf[:], 0.0)
        PIECE = 16
        for p0 in range(0, NWIN, PIECE):
            pw = min(PIECE, NWIN - p0)
            hrp = sb.tile([10, pw * P], f32, tag="hrp")
            nc.sync.dma_start(hrp[:], t_hrT[:, p0 * P:(p0 + pw) * P])
            for wl in range(pw):
                w = p0 + wl
                psz = ps3p.tile([P, 64], f32, tag="p3")
                nc.tensor.matmul(psz[:], lhsT=hrp[:, wl * P:(wl + 1) * P],
                                 rhs=w0_sb[:], start=True, stop=True)
                nc.scalar.mul(bigbuf[:, w, :64], psz[:], rs_r[:, w:w + 1])
        nc.sync.dma_start(
            TA[0][:].rearrange("(w p) f -> p w f", p=P), bigbuf[:])

        # ---------- pass machinery ----------
        gq_counter = [0]  # global SWDGE-DMA issue counter (queue rotation)

        def gather_plan(blk, half_split_blocks):
            """split blocks into dma_gather calls of <=GB blocks, not
            crossing the half boundary (in block index space)."""
            calls = []
            for lo, hi in half_split_blocks:
                b = lo
                while b < hi:
                    n = min(GB, hi - b)
                    calls.append((b, n))
                    b += n
            return calls

        def scatter_pass(idx_dram, drel_dram, nblk, blk, tables, on_group,
                         width=P):
            """Generic pass: gather + one-hot + psum accumulate per group.

            tables: list of (in_ap, blocks_lo, blocks_hi) gather sources.
            on_group(g, ps1): consume the accumulated psum for group g.
            width: valid feature columns in the gathered rows (matmul N).
            """
            idx_sb = cst.tile([P, max(nblkA, nblkB) * 8], i16, tag="idxsb")
            nc.sync.dma_start(idx_sb[:, :nblk * 8], idx_dram[:])
            drel_sb = cst.tile([P, max(nblkA, nblkB)], bf16, tag="drelsb")
            nc.sync.dma_start(drel_sb[:, :nblk], drel_dram[:])

            calls = gather_plan(blk, [(lo, hi) for _, lo, hi in tables])
            tbl_of_call = {}
            for ci, (b0, nb) in enumerate(calls):
                for ap, lo, hi in tables:
                    if lo <= b0 < hi:
                        tbl_of_call[ci] = ap
            call_of_block = {}
            for ci, (b0, nb) in enumerate(calls):
                for b in range(b0, b0 + nb):
                    call_of_block[b] = ci

            gtiles, ohtiles = {}, {}
            starts = np.concatenate([[0], np.cumsum(blk)]).astype(int)
            ngrp = len(blk)
            for g in range(ngrp):
                ps1 = ps1p.tile([P, P], f32, tag="p1")
                for b in range(starts[g], starts[g + 1]):
                    ci = call_of_block[b]
                    if ci not in gtiles:
                        b0, nb = calls[ci]
                        gt = gpool.tile([P, nb, P], bf16, tag="gbuf")
                        # Round-robin the 4 SWDGE queues so descriptor
                        # generation parallelizes across Q7 core pairs.
                        # queue = counter%4 with Tile's DMASW lane =
                        # counter%8 keeps lane-sharing gathers on one
                        # queue (FIFO), so lane sems stay ordered.
                        nc.gpsimd.dma_gather(
                            gt[:], tbl_of_call[ci], idx_sb[:, b0 * 8:
                                                          (b0 + nb) * 8],
                            nb * P, nb * P, P, single_packet=False,
                            queue_num=gq_counter[0] % 4)
                        gq_counter[0] += 1
                        gtiles[ci] = gt
                        # drop old refs so pool slots rotate
                        for k in list(gtiles):
                            if k < ci - 5:
                                del gtiles[k]
                    oi = b // GRP
                    if oi not in ohtiles:
                        o0 = oi * GRP
                        on = min(GRP, nblk - o0)
                        oh = ohpool.tile([P, P, on], bf16, tag="oh")
                        nc.vector.tensor_tensor(
                            out=oh[:],
                            in0=drel_sb[:, o0:o0 + on].unsqueeze(1)
                                .to_broadcast([P, P, on]),
                            in1=iota_rep[:, :, :on],
                            op=mybir.AluOpType.is_equal)
                        ohtiles[oi] = oh
                        for k in list(ohtiles):
                            if k < oi - 1:
                                del ohtiles[k]
                    b0, nb = calls[ci]
                    nc.tensor.matmul(
                        ps1[:, :width],
                        lhsT=ohtiles[oi][:, :, b - oi * GRP],
                        rhs=gtiles[ci][:, b - b0, :width],
                        start=(b == starts[g]),
                        stop=(b == starts[g + 1] - 1))
                on_group(g, ps1)

        # ---------- intron-side (order A) pass: l in {0,2,4} ----------
        def a_pass(l, ta_ap):
            astage = [None]
            aw = AW[l]

            def shard_compute(h):
                zstage = None
                for cc in range(NSC):
                    sh_in = sb.tile([P, aw], bf16, tag=f"shin{aw}")
                    nc.sync.dma_start(
                        sh_in[:], rsout[l][h][cc * P:(cc + 1) * P, :])
                    col = h * NSC + cc
                    if cc % 4 == 0:
                        zstage = sb.tile([P, 4, P], bf16, tag="zst")
                        if l == 0:
                            # top half of TB[0] rows stays zero (z1 is
                            # 64-wide); sentinel rows rely on zero agg+bias
                            nc.vector.memset(zstage[:, :, 64:], 0.0)
                    if l == 0:
                        nc.scalar.activation(
                            zstage[:, cc % 4, :64], sh_in[:],
                            mybir.ActivationFunctionType.Relu,
                            scale=r2g[l][:, col:col + 1])
                    else:
                        ps2 = ps2p.tile([P, P], bf16, tag="p2")
                        nc.tensor.transpose(ps2[:], sh_in[:], ident_b[:])
                        uT = uT128[cc % 2]
                        nc.scalar.copy(uT[:], ps2[:])
                        ps3 = ps3p.tile([P, P], f32, tag="p3")
                        nc.tensor.matmul(ps3[:], lhsT=uT[:], rhs=w_sb[l][:],
                                         start=True, stop=False)
                        nc.tensor.matmul(ps3[:], lhsT=ones_b[:],
                                         rhs=b_sb[l][:],
                                         start=False, stop=True)
                        nc.scalar.activation(
                            zstage[:, cc % 4, :], ps3[:],
                            mybir.ActivationFunctionType.Relu,
                            scale=r2g[l][:, col:col + 1])
                    if cc % 4 == 3 or cc == NSC - 1:
                        c0 = cc - cc % 4
                        nc.sync.dma_start(
                            zsh[l][h][c0 * P:(cc + 1) * P, :]
                            .rearrange("(c p) f -> p c f", p=P),
                            zstage[:, :cc % 4 + 1, :])

            def on_group(ch, ps1):
                if ch % 4 == 0:
                    astage[0] = sb.tile([P, 4, aw], bf16, tag=f"ast{aw}",
                                        name="ast")
                nc.scalar.copy(astage[0][:, ch % 4, :], ps1[:, :aw])
                if ch % 4 == 3:
                    h, chl = ch // NWIN, (ch - ch // NWIN * NWIN)
                    c0 = chl - 3
                    nc.sync.dma_start(
                        rsin[l][h][c0 * P:(chl + 1) * P, :]
                        .rearrange("(c p) f -> p c f", p=P), astage[0][:])

            scatter_pass(t_gidxA[:], t_drelA[:], nblkA, blkA,
                         [(ta_ap, 0, nblkA)], on_group, width=aw)
            # SWDGE gathers in flight concurrently with ncfw collectives
            # deadlock on this stack -- hard-serialize the pass tail.
            tc.strict_bb_all_engine_barrier()
            # issue both ReduceScatters up front: RS(1) runs on the
            # collective engines while shard_compute(0) runs on PE/ACT.
            for h in range(2):
                nc.gpsimd.collective_compute(
                    "ReduceScatter", mybir.AluOpType.add,
                    replica_groups=groups,
                    ins=[rsin[l][h][:].opt()], outs=[rsout[l][h][:].opt()])
            for h in range(2):
                shard_compute(h)
                nc.gpsimd.collective_compute(
                    "AllGather", mybir.AluOpType.bypass,
                    replica_groups=groups,
                    ins=[zsh[l][h][:].opt()], outs=[TB[l][h][:].opt()])
            tc.strict_bb_all_engine_barrier()

        # ---------- read-side (order B) pass: l in {1,3,5} ----------
        bstartsB = np.concatenate([[0], np.cumsum(blkB)]).astype(int)
        half_split_B = int(bstartsB[NWIN])

        def b_pass(l, out_sink):
            def post_window(w):
                fin = 64 if l == 1 else P
                ps2 = ps2p.tile([fin, P], bf16, tag="p2")
                nc.tensor.transpose(ps2[:], bigbuf[:, w, :fin], ident_b[:])
                if l == 1:
                    uT = uT65[w % 2]
                    nc.scalar.copy(uT[:64, :], ps2[:])
                    ps3 = ps3p.tile([P, P], f32, tag="p3")
                    nc.tensor.matmul(ps3[:], lhsT=uT[:], rhs=wext1_sb[:],
                                     start=True, stop=True)
                else:
                    uT = uT128[w % 2]
                    nc.scalar.copy(uT[:], ps2[:])
                    ps3 = ps3p.tile([P, P], f32, tag="p3")
                    nc.tensor.matmul(ps3[:], lhsT=uT[:], rhs=w_sb[l][:],
                                     start=True, stop=False)
                    nc.tensor.matmul(ps3[:], lhsT=ones_b[:], rhs=b_sb[l][:],
                                     start=False, stop=True)
                out_sink(w, ps3)

            bw = 64 if l == 1 else P

            def on_group(g, ps1):
                h, w = g // NWIN, g % NWIN
                if h == 0:
                    nc.scalar.copy(bigbuf[:, w, :bw], ps1[:, :bw])
                else:
                    nc.vector.tensor_tensor(
                        out=bigbuf[:, w, :bw], in0=ps1[:, :bw],
                        in1=bigbuf[:, w, :bw],
                        op=mybir.AluOpType.add)
                    post_window(w)

            tabs = [(TB[l - 1][0][:], 0, half_split_B),
                    (TB[l - 1][1][:], half_split_B, nblkB)]
            scatter_pass(t_gidxB[:], t_drelB[:], nblkB, blkB, tabs, on_group,
                         width=bw)

        # L0
        a_pass(0, TA[0][:])

        # L1 -> TA[1]
        zr_stage = [None]

        def sink_l1(w, ps3, l=1, k=1):
            if w % 4 == 0:
                zr_stage[0] = sb.tile([P, 4, P], bf16, tag="zrst", name="zrst")
            nc.scalar.activation(zr_stage[0][:, w % 4, :], ps3[:],
                                 mybir.ActivationFunctionType.Relu,
                                 scale=rgr[l][:, w:w + 1])
            if w % 4 == 3:
                c0 = w - 3
                nc.sync.dma_start(
                    TA[k][c0 * P:(w + 1) * P, :]
                    .rearrange("(c p) f -> p c f", p=P), zr_stage[0][:])

        b_pass(1, sink_l1)
        a_pass(2, TA[1][:])
        b_pass(3, lambda w, ps3: sink_l1(w, ps3, l=3, k=2))
        a_pass(4, TA[2][:])

        # L5 -> fc -> out
        out_sb = cst.tile([P, NWIN, 2], f32, tag="outsb")

        def sink_l5(w, ps3):
            h5 = sb.tile([P, P], f32, tag="h5")
            nc.scalar.activation(h5[:], ps3[:],
                                 mybir.ActivationFunctionType.Relu,
                                 scale=rgr[5][:, w:w + 1])
            ps2b = ps2p.tile([P, P], f32, tag="p2")
            nc.tensor.transpose(ps2b[:], h5[:], ident_f[:])
            h5T = uT128[w % 2]
            nc.scalar.copy(h5T[:], ps2b[:])
            psf = ps3p.tile([P, 2], f32, tag="p3")
            nc.tensor.matmul(psf[:], lhsT=h5T[:], rhs=fcw_sb[:],
                             start=True, stop=False)
            nc.tensor.matmul(psf[:], lhsT=ones_b[:], rhs=fcb_sb[:],
                             start=False, stop=True)
            nc.scalar.copy(out_sb[:, w, :], psf[:])

        b_pass(5, sink_l5)
        nc.sync.dma_start(
            t_out[:].rearrange("(w p) c -> p w c", p=P), out_sb[:])

    nc.compile()
    return nc


# ----------------------------------------------------------------------
# entry point
# ----------------------------------------------------------------------

def _ensure_ntff_hook():
    """Install the axon NTFF profiling hook shim if the image's antenv
    lacks the axon_hooks module (profiling-only; safe to fail)."""
    try:
        from antenv.axon_hooks import get_axon_ntff_profile_hook
        return get_axon_ntff_profile_hook() is not None
    except ImportError:
        pass
    try:
        import types
        import antenv
        from trn_agent_boot.trn_boot import _ntff_profile_via_ctypes
        mod = types.ModuleType("antenv.axon_hooks")
        mod._hook = _ntff_profile_via_ctypes("/opt/axon/libaxon_pjrt.so")
        mod.get_axon_ntff_profile_hook = lambda: mod._hook
        mod.set_axon_ntff_profile_hook = (
            lambda h: setattr(mod, "_hook", h))
        sys.modules["antenv.axon_hooks"] = mod
        antenv.axon_hooks = mod
        return mod._hook is not None
    except Exception:
        return False


def kernel(**inputs):
    global LAST_RESULTS
    in_maps, meta, rperms = _prep(inputs)
    if meta not in _BUILT:
        _BUILT[meta] = _build(meta)
    nc = _BUILT[meta]
    trace = bool(int(os.environ.get("BASS_TRACE", "0")))
    if trace:
        trace = _ensure_ntff_hook()
    if trace:
        # zero-egress container: keep profiling artifacts local
        bass_utils.upload_artifacts = lambda d: d
    try:
        res = bass_utils.run_bass_kernel_spmd(
            nc, in_maps, core_ids=list(range(NCORES)), trace=trace)
    except Exception:
        if not trace:
            raise
        os.environ["BASS_NEVER_TRACE"] = "1"
        res = bass_utils.run_bass_kernel_spmd(
            nc, in_maps, core_ids=list(range(NCORES)), trace=False)
    LAST_RESULTS = res
    out = np.empty((N_READ, 2), np.float32)
    for c in range(NCORES):
        out[c * R_LOC:(c + 1) * R_LOC] = res.results[c]["out"][rperms[c]]
    return out



# revision 23
# speedup vs baseline: 3.2569x; 1.0475x over previous
"""Trainium2 Bass kernel for nn_BipartiteGCN (6-layer bipartite GCN,
200K read nodes, 50K intron nodes, 2M random edges).

Strategy (8 NeuronCores, SPMD):
 - Shard edges by READ-node range: core c owns reads [25000c, 25000(c+1))
   and ALL edges incident to them.  Read-side aggregations are then exact
   and local; intron-side aggregations produce partials that are combined
   with ReduceScatter (+AllGather of the processed feature tables).
 - Per layer, node features live in DRAM tables of 256B bf16 rows
   (feature dim padded to 128).  The random side of each layer's
   gather/scatter is handled by batched SWDGE dma_gather (256B rows,
   int16 indices, tables kept < 32768 rows by splitting the intron table
   into two halves); the sorted side is handled by one-hot matmuls on the
   TensorEngine accumulating segment sums in PSUM.
 - One-hots are built on VectorE as bf16 is_equal against a pre-built
   replicated iota, 32 blocks (4096 edges) per instruction.
 - Degree vectors (pure functions of the integer edge lists, like the
   sort/padding metadata itself) are computed host-side as bincounts; the
   float math 1/sqrt(max(deg,1)) and everything downstream runs on device.
 - D^-1/2 scalings are folded into table construction (source side) and
   into per-partition activation scales at PSUM-drain time (dest side,
   using relu(s*x) = s*relu(x) for s>0).
"""

import os
import sys
import numpy as np
import ml_dtypes

sys.path.insert(0, "/opt/trn_rl_repo")

from contextlib import ExitStack

import concourse.bass as bass
import concourse.tile as tile
import concourse.mybir as mybir
from concourse import bacc, bass_utils
from concourse.masks import make_identity

P = 128
NCORES = 8
N_READ = 200000
N_INTRON = 50000
N_EDGES = 2000000

R_LOC = 25000          # real reads per core
R_PAD = 25600          # padded read slots per core (200 windows)
NWIN = 200             # read windows (chunks of 128); == NCH_A//2
ISLOT = 51200          # padded intron slots (400 chunks)
NCH_A = 400            # intron chunks
IHALF = 25600          # intron slots per half (== RSROWS: no tail pad)
RSROWS = 25600         # RS payload rows per half (divisible: 3200/rank)
SHARD = 3200           # rows per rank after RS
NSC = 25               # shard chunks of 128 rows
SENT_A = R_PAD - 1     # sentinel gather row in TA tables (kept-empty slot)
SENT_B = IHALF - 1     # sentinel gather row in TB tables (kept-empty slot)
GRP = 32               # blocks per one-hot instruction
GB = 32                # blocks per dma_gather call

bf16 = mybir.dt.bfloat16
f32 = mybir.dt.float32
i16 = mybir.dt.int16

_BUILT = {}
LAST_RESULTS = None


# ----------------------------------------------------------------------
# host-side prep
# ----------------------------------------------------------------------

def _wrap_idx(a):
    """[N] -> [128, N//16] int16 wrapped layout for dma_gather."""
    w = a.astype(np.int16).reshape(-1, 16).T
    return np.ascontiguousarray(np.tile(w, (8, 1)))


def _pmaj(a, nblk):
    """flat [nblk*128] -> [128, nblk] partition-major (e -> [e%128, e//128])"""
    return np.ascontiguousarray(a.reshape(nblk, P).T)


def _build_order(gid, payload_idx, payload_rel, n_groups, blk):
    """Scatter edges (sorted by group id, then ascending gather idx for
    DRAM locality in the SDMA drain) into padded per-group block slots.

    gid: [n] group id per edge;  blk: [n_groups] blocks per group (shared
    across cores).  Returns (gidx_pad int64, rel_pad int64) of length
    sum(blk)*128 with sentinel -1 in unfilled slots."""
    order = np.lexsort((payload_idx, gid))
    gs = gid[order]
    cnt = np.bincount(gid, minlength=n_groups)
    raw_start = np.concatenate([[0], np.cumsum(cnt)[:-1]])
    pad_start = np.concatenate([[0], np.cumsum(blk)[:-1]]) * P
    n = gid.shape[0]
    pos_in_g = np.arange(n) - np.repeat(raw_start, cnt)
    pos = pad_start[gs] + pos_in_g
    tot = int(blk.sum()) * P
    gidx = np.full(tot, -1, np.int64)
    rel = np.zeros(tot, np.int64)
    gidx[pos] = payload_idx[order]
    rel[pos] = payload_rel[order]
    return gidx, rel


def _pack_bins(w, nbins, caps):
    """Greedy multi-dim bin packing: place items (rows of w, heaviest
    total first) into nbins bins minimizing the max per-dimension bin sum,
    respecting per-bin item capacity caps.  Returns bin id per item."""
    order = np.argsort(-w.sum(1), kind="stable")
    sums = np.zeros((nbins, w.shape[1]))
    cnt = np.zeros(nbins, np.int64)
    asg = np.empty(len(w), np.int64)
    for i in order:
        s = (sums + w[i]).max(1)
        s[cnt >= caps] = np.inf
        b = int(np.argmin(s))
        asg[i] = b
        sums[b] += w[i]
        cnt[b] += 1
    return asg


def _bins_to_slots(asg, nbins):
    """bin assignment -> slot id (bin*128 + position within bin)."""
    order = np.argsort(asg, kind="stable")
    sorted_bins = asg[order]
    start = np.searchsorted(sorted_bins, np.arange(nbins))
    pos = np.arange(len(asg)) - start[sorted_bins]
    slot = np.empty(len(asg), np.int64)
    slot[order] = sorted_bins * P + pos
    return slot


def _prep(inputs):
    src = np.asarray(inputs["edge_src"]).astype(np.int64)
    dst = np.asarray(inputs["edge_dst"]).astype(np.int64)
    h_read = np.asarray(inputs["h_read"]).astype(np.float32)

    core = src // R_LOC

    # --- balance introns into chunks (8-dim per-core degree balance) so
    # every (chunk, core) count fits 5 blocks of 128; keep the last slot
    # of each half empty (sentinel rows must stay zero).
    deg8 = np.zeros((N_INTRON, NCORES), np.int64)
    np.add.at(deg8, (dst, core), 1)
    capsA = np.full(NCH_A, P, np.int64)
    capsA[NCH_A // 2 - 1] = P - 1
    capsA[NCH_A - 1] = P - 1
    iperm = _bins_to_slots(_pack_bins(deg8.astype(np.float64), NCH_A, capsA),
                           NCH_A)
    d_slot_all = iperm[dst]
    deg_i_glob = np.bincount(d_slot_all, minlength=ISLOT).astype(np.float32)

    per_core = []
    rperms = []
    cntA = np.zeros((NCORES, NCH_A), np.int64)
    cntB = np.zeros((NCORES, 2 * NWIN), np.int64)
    for c in range(NCORES):
        m = core == c
        s_raw = src[m] - c * R_LOC
        d = d_slot_all[m]
        half = d // IHALF
        # balance this core's reads into windows (per-half degree balance);
        # keep the last slot (SENT_A) empty.
        w2 = np.zeros((R_LOC, 2), np.int64)
        np.add.at(w2, (s_raw, half), 1)
        capsB = np.full(NWIN, P, np.int64)
        capsB[NWIN - 1] = P - 1
        rperm = _bins_to_slots(_pack_bins(w2.astype(np.float64), NWIN, capsB),
                               NWIN)
        rperms.append(rperm)
        s = rperm[s_raw]
        chA = d // P                       # order-A group: intron chunk
        gB = half * NWIN + (s // P)        # order-B group: (half, window)
        cntA[c] = np.bincount(chA, minlength=NCH_A)
        cntB[c] = np.bincount(gB, minlength=2 * NWIN)
        per_core.append((s, d, chA, gB, half))

    blkA = np.maximum(1, -(-cntA.max(axis=0) // P))
    blkB = np.maximum(1, -(-cntB.max(axis=0) // P))
    nblkA = int(blkA.sum())
    nblkB = int(blkB.sum())

    in_maps = []
    for c in range(NCORES):
        s, d, chA, gB, half = per_core[c]
        gA, relA = _build_order(chA, s, d % P, NCH_A, blkA)
        gA[gA < 0] = SENT_A
        gBi, relB = _build_order(gB, d - half * IHALF, s % P, 2 * NWIN, blkB)
        gBi[gBi < 0] = SENT_B

        deg_r = np.bincount(s, minlength=R_PAD).astype(np.float32)
        # my shard slice of intron degrees: col j = h*NSC + cc;
        # slot = h*IHALF + c*SHARD + cc*128 + p  (junk rows -> 1.0)
        deg_my = np.ones((P, 2 * NSC), np.float32)
        for h in range(2):
            rows = c * SHARD + np.arange(NSC * P)
            valid = rows < IHALF
            slots = h * IHALF + rows
            v = np.ones(NSC * P, np.float32)
            v[valid] = deg_i_glob[slots[valid]]
            deg_my[:, h * NSC:(h + 1) * NSC] = v.reshape(NSC, P).T

        hrT = np.zeros((10, R_PAD), np.float32)
        hrT[:, rperms[c]] = h_read[c * R_LOC:(c + 1) * R_LOC].T

        bf = ml_dtypes.bfloat16
        im = {
            "gidxA": _wrap_idx(gA),
            "drelA": _pmaj(relA, nblkA).astype(bf),
            "gidxB": _wrap_idx(gBi),
            "drelB": _pmaj(relB, nblkB).astype(bf),
            "hrT": hrT,
            "degr": np.ascontiguousarray(
                deg_r.reshape(NWIN, P).T).astype(np.float32),
            "degi": deg_my,
            "w0": np.asarray(inputs["W0"]).astype(np.float32),
            "wext1": np.concatenate(
                [np.asarray(inputs["W1"]),
                 np.asarray(inputs["b1"])[None, :]], 0).astype(bf),
            "atts": np.asarray(inputs["atts"]).reshape(1, 6)
                      .astype(np.float32),
            "fcw": np.asarray(inputs["fc_w"]).astype(bf),
            "fcb": np.asarray(inputs["fc_b"]).reshape(1, 2).astype(bf),
        }
        for l in (2, 3, 4, 5):
            im[f"w{l}"] = np.asarray(inputs[f"W{l}"]).astype(bf)
            im[f"b{l}"] = np.asarray(inputs[f"b{l}"]).reshape(1, P).astype(bf)
        in_maps.append(im)

    meta = (tuple(int(x) for x in blkA), tuple(int(x) for x in blkB))
    return in_maps, meta, rperms


# ----------------------------------------------------------------------
# device program
# ----------------------------------------------------------------------

def _build(meta):
    blkA, blkB = (np.array(meta[0]), np.array(meta[1]))
    nblkA, nblkB = int(blkA.sum()), int(blkB.sum())

    nc = bacc.Bacc("TRN2", target_bir_lowering=False, debug=False,
                   num_devices=NCORES, num_swdge_queues=4)

    # --- I/O ---
    t_gidxA = nc.dram_tensor("gidxA", [P, nblkA * 8], i16, kind="ExternalInput")
    t_drelA = nc.dram_tensor("drelA", [P, nblkA], bf16, kind="ExternalInput")
    t_gidxB = nc.dram_tensor("gidxB", [P, nblkB * 8], i16, kind="ExternalInput")
    t_drelB = nc.dram_tensor("drelB", [P, nblkB], bf16, kind="ExternalInput")
    t_hrT = nc.dram_tensor("hrT", [10, R_PAD], f32, kind="ExternalInput")
    t_degr = nc.dram_tensor("degr", [P, NWIN], f32, kind="ExternalInput")
    t_degi = nc.dram_tensor("degi", [P, 2 * NSC], f32, kind="ExternalInput")
    t_w0 = nc.dram_tensor("w0", [10, 64], f32, kind="ExternalInput")
    t_wext1 = nc.dram_tensor("wext1", [65, P], bf16, kind="ExternalInput")
    t_w = {l: nc.dram_tensor(f"w{l}", [P, P], bf16, kind="ExternalInput")
           for l in (2, 3, 4, 5)}
    t_b = {l: nc.dram_tensor(f"b{l}", [1, P], bf16, kind="ExternalInput")
           for l in (2, 3, 4, 5)}
    t_fcw = nc.dram_tensor("fcw", [P, 2], bf16, kind="ExternalInput")
    t_fcb = nc.dram_tensor("fcb", [1, 2], bf16, kind="ExternalInput")
    t_atts = nc.dram_tensor("atts", [1, 6], f32, kind="ExternalInput")
    t_out = nc.dram_tensor("out", [R_PAD, 2], f32, kind="ExternalOutput")

    groups = [list(range(NCORES))]

    with tile.TileContext(nc) as tc, ExitStack() as ctx:
        cst = ctx.enter_context(tc.tile_pool(name="cst", bufs=1))
        sb = ctx.enter_context(tc.tile_pool(name="sb", bufs=3))
        gpool = ctx.enter_context(tc.tile_pool(name="gp", bufs=6))
        ohpool = ctx.enter_context(tc.tile_pool(name="oh", bufs=2))
        ps1p = ctx.enter_context(tc.tile_pool(name="ps1", bufs=3, space="PSUM"))
        ps2p = ctx.enter_context(tc.tile_pool(name="ps2", bufs=2, space="PSUM"))
        ps3p = ctx.enter_context(tc.tile_pool(name="ps3", bufs=3, space="PSUM"))
        dram = ctx.enter_context(tc.tile_pool(name="dr", bufs=1, space="DRAM"))

        # ---------- constants ----------
        iota_i = cst.tile([P, P], mybir.dt.int32)
        nc.gpsimd.iota(iota_i[:], pattern=[[1, P]], base=0,
                       channel_multiplier=0)
        iota_bf = cst.tile([P, P], bf16)
        nc.vector.tensor_copy(iota_bf[:], iota_i[:])
        iota_rep = cst.tile([P, P, GRP], bf16)
        nc.vector.tensor_copy(
            iota_rep[:], iota_bf[:].unsqueeze(2).to_broadcast([P, P, GRP]))

        ident_f = cst.tile([P, P], f32)
        make_identity(nc, ident_f[:])
        ident_b = cst.tile([P, P], bf16)
        nc.vector.tensor_copy(ident_b[:], ident_f[:])

        ones_f = cst.tile([1, P], f32)
        nc.vector.memset(ones_f[:], 1.0)
        ones_b = cst.tile([1, P], bf16)
        nc.vector.memset(ones_b[:], 1.0)

        # weights
        w0_sb = cst.tile([10, 64], f32)
        nc.sync.dma_start(w0_sb[:], t_w0[:])
        wext1_sb = cst.tile([65, P], bf16)
        nc.sync.dma_start(wext1_sb[:], t_wext1[:])
        w_sb, b_sb = {}, {}
        for l in (2, 3, 4, 5):
            w_sb[l] = cst.tile([P, P], bf16, tag=f"w{l}", name=f"w{l}sb")
            nc.sync.dma_start(w_sb[l][:], t_w[l][:])
            b_sb[l] = cst.tile([1, P], bf16, tag=f"b{l}", name=f"b{l}sb")
            nc.sync.dma_start(b_sb[l][:], t_b[l][:])
        fcw_sb = cst.tile([P, 2], bf16)
        nc.sync.dma_start(fcw_sb[:], t_fcw[:])
        fcb_sb = cst.tile([1, 2], bf16)
        nc.sync.dma_start(fcb_sb[:], t_fcb[:])

        # gates: sigmoid(atts) replicated to 128 partitions
        atts_sb = cst.tile([1, 6], f32)
        nc.sync.dma_start(atts_sb[:], t_atts[:])
        sg = cst.tile([1, 6], f32)
        nc.scalar.activation(sg[:], atts_sb[:],
                             mybir.ActivationFunctionType.Sigmoid)
        ps_g = ps3p.tile([P, 6], f32, tag="p3")
        nc.tensor.matmul(ps_g[:], lhsT=ones_f[:], rhs=sg[:],
                         start=True, stop=True)
        g_rep = cst.tile([P, 6], f32)
        nc.scalar.copy(g_rep[:], ps_g[:])

        # rs_r = 1/sqrt(max(deg_r,1)); per-layer drain scales
        degr_sb = cst.tile([P, NWIN], f32)
        nc.sync.dma_start(degr_sb[:], t_degr[:])
        rs_r = cst.tile([P, NWIN], f32)
        nc.vector.tensor_scalar_max(rs_r[:], degr_sb[:], 1.0)
        nc.scalar.sqrt(rs_r[:], rs_r[:])
        nc.vector.reciprocal(rs_r[:], rs_r[:])
        rgr = {}
        for l in (1, 3, 5):
            rgr[l] = cst.tile([P, NWIN], f32, tag=f"rgr{l}", name=f"rgr{l}")
            if l == 5:
                # last conv layer: no outer rs_r fold -> scale = rs_r * g5
                nc.vector.tensor_copy(rgr[l][:], rs_r[:])
            else:
                nc.vector.tensor_tensor(out=rgr[l][:], in0=rs_r[:],
                                        in1=rs_r[:], op=mybir.AluOpType.mult)
            nc.vector.tensor_tensor(
                out=rgr[l][:], in0=rgr[l][:],
                in1=g_rep[:, l:l + 1].to_broadcast([P, NWIN]),
                op=mybir.AluOpType.mult)

        degi_sb = cst.tile([P, 2 * NSC], f32)
        nc.sync.dma_start(degi_sb[:], t_degi[:])
        rs_i = cst.tile([P, 2 * NSC], f32)
        nc.vector.tensor_scalar_max(rs_i[:], degi_sb[:], 1.0)
        nc.scalar.sqrt(rs_i[:], rs_i[:])
        nc.vector.reciprocal(rs_i[:], rs_i[:])
        r2g = {}
        for l in (0, 2, 4):
            r2g[l] = cst.tile([P, 2 * NSC], f32, tag=f"r2g{l}", name=f"r2g{l}")
            nc.vector.tensor_tensor(out=r2g[l][:], in0=rs_i[:], in1=rs_i[:],
                                    op=mybir.AluOpType.mult)
            nc.vector.tensor_tensor(
                out=r2g[l][:], in0=r2g[l][:],
                in1=g_rep[:, l:l + 1].to_broadcast([P, 2 * NSC]),
                op=mybir.AluOpType.mult)

        # uT staging buffers with a fixed ones row (fin=64 path);
        # 6-deep rotation breaks the transpose->copy->matmul latency chain
        NUT = 6
        uT65 = [cst.tile([65, P], bf16, tag=f"uT65_{i}", name=f"uT65_{i}")
                for i in range(NUT)]
        for t in uT65:
            nc.vector.memset(t[:], 1.0)
        uT128 = [cst.tile([P, P], bf16, tag=f"uT128_{i}", name=f"uT128_{i}")
                 for i in range(NUT)]

        # big shared buffer: z0 staging / order-B aggregation
        bigbuf = cst.tile([P, NWIN, P], bf16, tag="bigbuf")

        # DRAM tables & collective buffers
        TA = [dram.tile([R_PAD, P], bf16, tag=f"TA{k}", name=f"TA{k}")
              for k in range(3)]
        TB = {}
        rsin, rsout, zsh = {}, {}, {}
        AW = {0: 64, 2: P, 4: P}   # aggregation width per a-layer
        for l in (0, 2, 4):
            TB[l] = [dram.tile([RSROWS, P], bf16, tag=f"TB{l}_{h}",
                               name=f"TB{l}_{h}",
                               addr_space="Shared") for h in range(2)]
            rsin[l] = [dram.tile([RSROWS, AW[l]], bf16, tag=f"rsin{l}_{h}",
                                 name=f"rsin{l}_{h}")
                       for h in range(2)]
            rsout[l] = [dram.tile([SHARD, AW[l]], bf16, tag=f"rso{l}_{h}",
                                  name=f"rso{l}_{h}")
                        for h in range(2)]
            zsh[l] = [dram.tile([SHARD, P], bf16, tag=f"zsh{l}_{h}",
                                name=f"zsh{l}_{h}")
                      for h in range(2)]

        # ---------- z0 = (h_read * rs_r) @ W0  ->  TA[0] ----------
        nc.vector.memset(bigbuf[:], 0.0)
        PIECE = 8
        for p0 in range(0, NWIN, PIECE):
            pw = min(PIECE, NWIN - p0)
            hrp = sb.tile([10, pw * P], f32, tag="hrp")
            nc.sync.dma_start(hrp[:], t_hrT[:, p0 * P:(p0 + pw) * P])
            for wl in range(pw):
                w = p0 + wl
                psz = ps3p.tile([P, 64], f32, tag="p3")
                nc.tensor.matmul(psz[:], lhsT=hrp[:, wl * P:(wl + 1) * P],
                                 rhs=w0_sb[:], start=True, stop=True)
                nc.scalar.mul(bigbuf[:, w, :64], psz[:], rs_r[:, w:w + 1])
        nc.sync.dma_start(
            TA[0][:].rearrange("(w p) f -> p w f", p=P), bigbuf[:])

        # ---------- pass machinery ----------
        gq_counter = [0]  # global SWDGE-DMA issue counter (queue rotation)

        def gather_plan(blk, half_split_blocks):
            """split blocks into dma_gather calls of <=GB blocks, not
            crossing the half boundary (in block index space)."""
            calls = []
            for lo, hi in half_split_blocks:
                b = lo
                while b < hi:
                    n = min(GB, hi - b)
                    calls.append((b, n))
                    b += n
            return calls

        def scatter_pass(idx_dram, drel_dram, nblk, blk, tables, on_group,
                         width=P):
            """Generic pass: gather + one-hot + psum accumulate per group.

            tables: list of (in_ap, blocks_lo, blocks_hi) gather sources.
            on_group(g, ps1): consume the accumulated psum for group g.
            width: valid feature columns in the gathered rows (matmul N).
            """
            idx_sb = cst.tile([P, max(nblkA, nblkB) * 8], i16, tag="idxsb")
            nc.sync.dma_start(idx_sb[:, :nblk * 8], idx_dram[:])
            drel_sb = cst.tile([P, max(nblkA, nblkB)], bf16, tag="drelsb")
            nc.sync.dma_start(drel_sb[:, :nblk], drel_dram[:])

            calls = gather_plan(blk, [(lo, hi) for _, lo, hi in tables])
            tbl_of_call = {}
            for ci, (b0, nb) in enumerate(calls):
                for ap, lo, hi in tables:
                    if lo <= b0 < hi:
                        tbl_of_call[ci] = ap
            call_of_block = {}
            for ci, (b0, nb) in enumerate(calls):
                for b in range(b0, b0 + nb):
                    call_of_block[b] = ci

            gtiles, ohtiles = {}, {}
            starts = np.concatenate([[0], np.cumsum(blk)]).astype(int)
            ngrp = len(blk)
            for g in range(ngrp):
                ps1 = ps1p.tile([P, P], f32, tag="p1")
                for b in range(starts[g], starts[g + 1]):
                    ci = call_of_block[b]
                    if ci not in gtiles:
                        b0, nb = calls[ci]
                        gt = gpool.tile([P, nb, P], bf16, tag="gbuf")
                        # Round-robin the 4 SWDGE queues so descriptor
                        # generation parallelizes across Q7 core pairs.
                        # queue = counter%4 with Tile's DMASW lane =
                        # counter%8 keeps lane-sharing gathers on one
                        # queue (FIFO), so lane sems stay ordered.
                        nc.gpsimd.dma_gather(
                            gt[:], tbl_of_call[ci], idx_sb[:, b0 * 8:
                                                          (b0 + nb) * 8],
                            nb * P, nb * P, P, single_packet=False,
                            queue_num=gq_counter[0] % 4)
                        gq_counter[0] += 1
                        gtiles[ci] = gt
                        # drop old refs so pool slots rotate
                        for k in list(gtiles):
                            if k < ci - 5:
                                del gtiles[k]
                    oi = b // GRP
                    if oi not in ohtiles:
                        o0 = oi * GRP
                        on = min(GRP, nblk - o0)
                        oh = ohpool.tile([P, P, on], bf16, tag="oh")
                        nc.vector.tensor_tensor(
                            out=oh[:],
                            in0=drel_sb[:, o0:o0 + on].unsqueeze(1)
                                .to_broadcast([P, P, on]),
                            in1=iota_rep[:, :, :on],
                            op=mybir.AluOpType.is_equal)
                        ohtiles[oi] = oh
                        for k in list(ohtiles):
                            if k < oi - 1:
                                del ohtiles[k]
                    b0, nb = calls[ci]
                    nc.tensor.matmul(
                        ps1[:, :width],
                        lhsT=ohtiles[oi][:, :, b - oi * GRP],
                        rhs=gtiles[ci][:, b - b0, :width],
                        start=(b == starts[g]),
                        stop=(b == starts[g + 1] - 1))
                on_group(g, ps1)

        # ---------- intron-side (order A) pass: l in {0,2,4} ----------
        def a_pass(l, ta_ap):
            astage = [None]
            aw = AW[l]

            def shard_compute(h):
                zstage = None
                for cc in range(NSC):
                    sh_in = sb.tile([P, aw], bf16, tag=f"shin{aw}")
                    nc.sync.dma_start(
                        sh_in[:], rsout[l][h][cc * P:(cc + 1) * P, :])
                    col = h * NSC + cc
                    if cc % 4 == 0:
                        zstage = sb.tile([P, 4, P], bf16, tag="zst")
                        if l == 0:
                            # top half of TB[0] rows stays zero (z1 is
                            # 64-wide); sentinel rows rely on zero agg+bias
                            nc.vector.memset(zstage[:, :, 64:], 0.0)
                    if l == 0:
                        nc.scalar.activation(
                            zstage[:, cc % 4, :64], sh_in[:],
                            mybir.ActivationFunctionType.Relu,
                            scale=r2g[l][:, col:col + 1])
                    else:
                        ps2 = ps2p.tile([P, P], bf16, tag="p2")
                        nc.tensor.transpose(ps2[:], sh_in[:], ident_b[:])
                        uT = uT128[cc % NUT]
                        nc.scalar.copy(uT[:], ps2[:])
                        ps3 = ps3p.tile([P, P], f32, tag="p3")
                        nc.tensor.matmul(ps3[:], lhsT=uT[:], rhs=w_sb[l][:],
                                         start=True, stop=False)
                        nc.tensor.matmul(ps3[:], lhsT=ones_b[:],
                                         rhs=b_sb[l][:],
                                         start=False, stop=True)
                        nc.scalar.activation(
                            zstage[:, cc % 4, :], ps3[:],
                            mybir.ActivationFunctionType.Relu,
                            scale=r2g[l][:, col:col + 1])
                    if cc % 4 == 3 or cc == NSC - 1:
                        c0 = cc - cc % 4
                        nc.sync.dma_start(
                            zsh[l][h][c0 * P:(cc + 1) * P, :]
                            .rearrange("(c p) f -> p c f", p=P),
                            zstage[:, :cc % 4 + 1, :])

            def on_group(ch, ps1):
                if ch % 4 == 0:
                    astage[0] = sb.tile([P, 4, aw], bf16, tag=f"ast{aw}",
                                        name="ast")
                nc.scalar.copy(astage[0][:, ch % 4, :], ps1[:, :aw])
                if ch % 4 == 3:
                    h, chl = ch // NWIN, (ch - ch // NWIN * NWIN)
                    c0 = chl - 3
                    nc.sync.dma_start(
                        rsin[l][h][c0 * P:(chl + 1) * P, :]
                        .rearrange("(c p) f -> p c f", p=P), astage[0][:])

            scatter_pass(t_gidxA[:], t_drelA[:], nblkA, blkA,
                         [(ta_ap, 0, nblkA)], on_group, width=aw)
            # SWDGE gathers in flight concurrently with ncfw collectives
            # deadlock on this stack -- hard-serialize the pass tail.
            tc.strict_bb_all_engine_barrier()
            # issue both ReduceScatters up front: RS(1) runs on the
            # collective engines while shard_compute(0) runs on PE/ACT.
            for h in range(2):
                nc.gpsimd.collective_compute(
                    "ReduceScatter", mybir.AluOpType.add,
                    replica_groups=groups,
                    ins=[rsin[l][h][:].opt()], outs=[rsout[l][h][:].opt()])
            for h in range(2):
                shard_compute(h)
                nc.gpsimd.collective_compute(
                    "AllGather", mybir.AluOpType.bypass,
                    replica_groups=groups,
                    ins=[zsh[l][h][:].opt()], outs=[TB[l][h][:].opt()])
            tc.strict_bb_all_engine_barrier()

        # ---------- read-side (order B) pass: l in {1,3,5} ----------
        bstartsB = np.concatenate([[0], np.cumsum(blkB)]).astype(int)
        half_split_B = int(bstartsB[NWIN])

        def b_pass(l, out_sink):
            def post_window(w):
                fin = 64 if l == 1 else P
                ps2 = ps2p.tile([fin, P], bf16, tag="p2")
                nc.tensor.transpose(ps2[:], bigbuf[:, w, :fin], ident_b[:])
                if l == 1:
                    uT = uT65[w % NUT]
                    nc.scalar.copy(uT[:64, :], ps2[:])
                    ps3 = ps3p.tile([P, P], f32, tag="p3")
                    nc.tensor.matmul(ps3[:], lhsT=uT[:], rhs=wext1_sb[:],
                                     start=True, stop=True)
                else:
                    uT = uT128[w % NUT]
                    nc.scalar.copy(uT[:], ps2[:])
                    ps3 = ps3p.tile([P, P], f32, tag="p3")
                    nc.tensor.matmul(ps3[:], lhsT=uT[:], rhs=w_sb[l][:],
                                     start=True, stop=False)
                    nc.tensor.matmul(ps3[:], lhsT=ones_b[:], rhs=b_sb[l][:],
                                     start=False, stop=True)
                out_sink(w, ps3)

            bw = 64 if l == 1 else P

            def on_group(g, ps1):
                h, w = g // NWIN, g % NWIN
                if h == 0:
                    nc.scalar.copy(bigbuf[:, w, :bw], ps1[:, :bw])
                else:
                    nc.vector.tensor_tensor(
                        out=bigbuf[:, w, :bw], in0=ps1[:, :bw],
                        in1=bigbuf[:, w, :bw],
                        op=mybir.AluOpType.add)
                    post_window(w)

            tabs = [(TB[l - 1][0][:], 0, half_split_B),
                    (TB[l - 1][1][:], half_split_B, nblkB)]
            scatter_pass(t_gidxB[:], t_drelB[:], nblkB, blkB, tabs, on_group,
                         width=bw)

        # L0
        a_pass(0, TA[0][:])

        # L1 -> TA[1]
        zr_stage = [None]

        def sink_l1(w, ps3, l=1, k=1):
            if w % 4 == 0:
                zr_stage[0] = sb.tile([P, 4, P], bf16, tag="zrst", name="zrst")
            nc.scalar.activation(zr_stage[0][:, w % 4, :], ps3[:],
                                 mybir.ActivationFunctionType.Relu,
                                 scale=rgr[l][:, w:w + 1])
            if w % 4 == 3:
                c0 = w - 3
                nc.sync.dma_start(
                    TA[k][c0 * P:(w + 1) * P, :]
                    .rearrange("(c p) f -> p c f", p=P), zr_stage[0][:])

        b_pass(1, sink_l1)
        a_pass(2, TA[1][:])
        b_pass(3, lambda w, ps3: sink_l1(w, ps3, l=3, k=2))
        a_pass(4, TA[2][:])

        # L5 -> fc -> out
        out_sb = cst.tile([P, NWIN, 2], f32, tag="outsb")

        def sink_l5(w, ps3):
            h5 = sb.tile([P, P], f32, tag="h5")
            nc.scalar.activation(h5[:], ps3[:],
                                 mybir.ActivationFunctionType.Relu,
                                 scale=rgr[5][:, w:w + 1])
            ps2b = ps2p.tile([P, P], f32, tag="p2")
            nc.tensor.transpose(ps2b[:], h5[:], ident_f[:])
            h5T = uT128[w % NUT]
            nc.scalar.copy(h5T[:], ps2b[:])
            psf = ps3p.tile([P, 2], f32, tag="p3")
            nc.tensor.matmul(psf[:], lhsT=h5T[:], rhs=fcw_sb[:],
                             start=True, stop=False)
            nc.tensor.matmul(psf[:], lhsT=ones_b[:], rhs=fcb_sb[:],
                             start=False, stop=True)
            nc.scalar.copy(out_sb[:, w, :], psf[:])

        b_pass(5, sink_l5)
        nc.sync.dma_start(
            t_out[:].rearrange("(w p) c -> p w c", p=P), out_sb[:])

    nc.compile()
    return nc


# ----------------------------------------------------------------------
# entry point
# ----------------------------------------------------------------------

def _ensure_ntff_hook():
    """Install the axon NTFF profiling hook shim if the image's antenv
    lacks the axon_hooks module (profiling-only; safe to fail)."""
    try:
        from antenv.axon_hooks import get_axon_ntff_profile_hook
        return get_axon_ntff_profile_hook() is not None
    except ImportError:
        pass
    try:
        import types
        import antenv
        from trn_agent_boot.trn_boot import _ntff_profile_via_ctypes
        mod = types.ModuleType("antenv.axon_hooks")
        mod._hook = _ntff_profile_via_ctypes("/opt/axon/libaxon_pjrt.so")
        mod.get_axon_ntff_profile_hook = lambda: mod._hook
        mod.set_axon_ntff_profile_hook = (
            lambda h: setattr(mod, "_hook", h))
        sys.modules["antenv.axon_hooks"] = mod
        antenv.axon_hooks = mod
        return mod._hook is not None
    except Exception:
        return False


def kernel(**inputs):
    global LAST_RESULTS
    in_maps, meta, rperms = _prep(inputs)
    if meta not in _BUILT:
        _BUILT[meta] = _build(meta)
    nc = _BUILT[meta]
    trace = bool(int(os.environ.get("BASS_TRACE", "0")))
    if trace:
        trace = _ensure_ntff_hook()
    if trace:
        # zero-egress container: keep profiling artifacts local
        bass_utils.upload_artifacts = lambda d: d
    try:
        res = bass_utils.run_bass_kernel_spmd(
            nc, in_maps, core_ids=list(range(NCORES)), trace=trace)
    except Exception:
        if not trace:
            raise
        os.environ["BASS_NEVER_TRACE"] = "1"
        res = bass_utils.run_bass_kernel_spmd(
            nc, in_maps, core_ids=list(range(NCORES)), trace=False)
    LAST_RESULTS = res
    out = np.empty((N_READ, 2), np.float32)
    for c in range(NCORES):
        out[c * R_LOC:(c + 1) * R_LOC] = res.results[c]["out"][rperms[c]]
    return out

